# revision 18
# baseline (speedup 1.0000x reference)
"""Trainium2 distributed kernel for nn_AdaptiveHopModel (gnn_message_passing).

Model: two masked edge-attention blocks over E=4096 edges (D=256, H=4), an
adaptive per-edge hop gate alpha = sigmoid(MLP(x)), blended output fed
through a small classifier; returns (logits [E,16], alpha.mean()).

Sharding: query-edge rows (E) split across 8 NeuronCores (512 rows/core).
Each core computes q/k/v projections for ITS OWN edges only; k^T and the
ones-augmented v are then AllGathered (fp16, rank-concat on the leading
axis) so every core holds full-E keys/values.  Activations are kept in a
transposed [feature, edge] layout so every x@W matmul maps directly onto
the TensorEngine.  The masked softmax streams: PE scoresT -> ScalarE exp
(PSUM->SBUF, fp16) -> DVE/GPSIMD mask-multiply -> PE attn@V with a
ones-column in V producing row-sums for free; normalization happens on
the small [65, e] output.

Masks are pre-transposed/tiled on the host as fp16 {0,1}; alpha.mean() is
assembled on the host from per-core partials; logits come back transposed
[16, 512] per core and are transposed on the host.
"""

import sys

sys.path.insert(0, "/opt/trn_rl_repo")

import numpy as np

import concourse.bass as bass
import concourse.bacc as bacc
import concourse.tile as tile
import concourse.mybir as mybir
from concourse.bass_utils import run_bass_kernel_spmd

F16 = mybir.dt.float16
F32 = mybir.dt.float32
AF = mybir.ActivationFunctionType
MULT = mybir.AluOpType.mult
ADD = mybir.AluOpType.add

SQRT_2_OVER_PI = 0.7978845608028654
GELU_C = 0.044715


class Cfg:
    def __init__(self, E=4096, n_cores=8, CS=3):
        self.E = E                      # total edges
        self.D = 256                    # edge feature dim
        self.H = 4                      # heads
        self.DH = 64                    # head dim
        self.NC = n_cores
        self.EL = E // n_cores          # local query rows per core
        self.NFC = E // 128             # f-chunks of 128 key rows
        self.NFL = self.EL // 128       # local f-chunks
        self.CS = CS                    # f-chunks per softmax stripe
        self.NCLS = 16
        self.DG = 128                   # gate hidden dim
        self.n_stripes = (self.NFC + CS - 1) // CS


def _emit_core_program(nc, tc, ctx, io, cfg, rep, dbg=False):
    """Emit the whole per-core program (one repetition)."""
    E, EL, NFC, NFL, CS = cfg.E, cfg.EL, cfg.NFC, cfg.NFL, cfg.CS
    R = f"r{rep}"
    RG = [list(range(cfg.NC))]

    konst = ctx.enter_context(tc.tile_pool(name=f"konst{R}", bufs=1))
    ps512 = ctx.enter_context(tc.tile_pool(name=f"ps512{R}", bufs=2, space="PSUM"))
    tmp = ctx.enter_context(tc.tile_pool(name=f"tmp{R}", bufs=2))
    small = ctx.enter_context(tc.tile_pool(name=f"small{R}", bufs=1))
    apool = ctx.enter_context(tc.tile_pool(name=f"attn{R}", bufs=1))
    scp = ctx.enter_context(tc.tile_pool(name=f"scp{R}", bufs=2, space="PSUM"))
    stream = ctx.enter_context(tc.tile_pool(name=f"stream{R}", bufs=3))

    def ktile(nm, shape, dtype):
        return konst.tile(shape, dtype, name=nm, tag=nm)

    def load_const(nm, shape, dtype):
        t = ktile(nm, shape, dtype)
        nc.sync.dma_start(t[:], io[nm][:])
        return t

    def load_rows(nm, src, nrow=128, nsplit=1):
        n = io[src].shape[0] // nrow
        ts = []
        for d in range(n):
            t = ktile(f"{nm}{d}", [nrow, io[src].shape[1]], io[src].dtype)
            if nsplit == 1:
                nc.sync.dma_start(t[:], io[src][nrow * d:nrow * (d + 1), :])
            else:
                step = io[src].shape[1] // nsplit
                for i in range(nsplit):
                    nc.sync.dma_start(
                        t[:, step * i:step * (i + 1)],
                        io[src][nrow * d:nrow * (d + 1), step * i:step * (i + 1)])
            ts.append(t)
        return ts

    # ---------------- constant loads ----------------
    xTl = load_rows("xTl", "xTl", nsplit=2)
    wqkv, wv_aug, bv_aug, wo, bo, bq, bk = [], [], [], [], [], [], []
    for a in range(2):
        wqkv.append(load_rows(f"wqkv{a}_", f"wqkv{a}", nsplit=2))
        wv_aug.append(load_rows(f"wv_aug{a}_", f"wv_aug{a}"))
        bv_aug.append(load_const(f"bv_aug{a}", [1, 260], F16))
        wo.append(load_rows(f"wo{a}_", f"wo{a}"))
        bo.append(load_const(f"bo{a}", [1, 256], F16))
        bqt, bkt = [], []
        for m in range(2):
            t = ktile(f"bq{a}_{m}", [128, 1], F32)
            nc.sync.dma_start(t[:], io[f"bqkv{a}"][128 * m:128 * (m + 1), :])
            bqt.append(t)
            t = ktile(f"bk{a}_{m}", [128, 1], F32)
            nc.sync.dma_start(t[:], io[f"bqkv{a}"][256 + 128 * m:256 + 128 * (m + 1), :])
            bkt.append(t)
        bq.append(bqt)
        bk.append(bkt)

    wg1 = load_rows("wg1_", "wg1")
    bg1 = load_const("bg1", [cfg.DG, 1], F32)
    wg2h = load_const("wg2h", [cfg.DG, 1], F16)
    bg2h = load_const("bg2h", [1, 1], F32)
    wc1 = load_rows("wc1_", "wc1")
    bc1 = load_rows("bc1_", "bc1")
    wc2 = load_rows("wc2_", "wc2")
    bc2 = load_const("bc2", [cfg.NCLS, 1], F32)

    ones_row = ktile("ones_row", [1, 128], F16)
    nc.vector.memset(ones_row[:], 1.0)

    def pstile():
        return ps512.tile([128, 512], F32, name="ps", tag="ps")

    # ---------------- per-attn q/k/v local + AllGather ----------------
    qT, kT, v_store = [], [], []
    for a in range(2):
        qTa = [apool.tile([128, EL], F16, name=f"qT{m}_{a}", tag=f"qT{m}_{a}")
               for m in range(2)]
        for m in range(2):
            ps = pstile()
            psv = ps[:, 0:EL]
            nc.tensor.matmul(psv, wqkv[a][0][:, 128 * m:128 * (m + 1)],
                             xTl[0][:], start=True, stop=False)
            nc.tensor.matmul(psv, wqkv[a][1][:, 128 * m:128 * (m + 1)],
                             xTl[1][:], start=False, stop=True)
            nc.vector.tensor_scalar(qTa[m][:], psv, bq[a][m][:], None, op0=ADD)
        qT.append(qTa)

        # k_local = kT[:, own edges]  [256, EL] fp16 (bias included)
        kloc = apool.tile([128, 2 * EL], F16, name=f"kloc_{a}", tag=f"kloc_{a}")
        for m in range(2):
            ps = pstile()
            psv = ps[:, 0:EL]
            nc.tensor.matmul(psv, wqkv[a][0][:, 256 + 128 * m:256 + 128 * (m + 1)],
                             xTl[0][:], start=True, stop=False)
            nc.tensor.matmul(psv, wqkv[a][1][:, 256 + 128 * m:256 + 128 * (m + 1)],
                             xTl[1][:], start=False, stop=True)
            nc.vector.tensor_scalar(kloc[:, EL * m:EL * (m + 1)], psv,
                                    bk[a][m][:], None, op0=ADD)
        # v_local (+ ones col, + bias) for own edges
        vloc = apool.tile([128, NFL * 260], F16, name=f"vloc_{a}", tag=f"vloc_{a}")
        for j in range(NFL):
            ps = pstile()
            psv = ps[:, 0:260]
            fsl = slice(128 * j, 128 * (j + 1))
            nc.tensor.matmul(psv, xTl[0][:, fsl], wv_aug[a][0][:], start=True, stop=False)
            nc.tensor.matmul(psv, xTl[1][:, fsl], wv_aug[a][1][:], start=False, stop=False)
            nc.tensor.matmul(psv, ones_row[:], bv_aug[a][:], start=False, stop=True)
            nc.vector.tensor_copy(vloc[:, 260 * j:260 * (j + 1)], psv)

        # bounce to DRAM + AllGather + unload to SBUF
        for m in range(2):
            nc.sync.dma_start(io[f"kin{a}"][128 * m:128 * (m + 1), :],
                              kloc[:, EL * m:EL * (m + 1)])
        for j in range(NFL):
            nc.sync.dma_start(io[f"vin{a}"][128 * j:128 * (j + 1), :],
                              vloc[:, 260 * j:260 * (j + 1)])
        nc.gpsimd.collective_compute(
            "AllGather", mybir.AluOpType.bypass, replica_groups=RG,
            ins=[io[f"kin{a}"].opt()], outs=[io[f"kout{a}"].opt()])
        nc.gpsimd.collective_compute(
            "AllGather", mybir.AluOpType.bypass, replica_groups=RG,
            ins=[io[f"vin{a}"].opt()], outs=[io[f"vout{a}"].opt()])

        kTa = [apool.tile([128, E], F16, name=f"kT{m}_{a}", tag=f"kT{m}_{a}")
               for m in range(2)]
        for r in range(cfg.NC):
            for m in range(2):
                nc.sync.dma_start(
                    kTa[m][:, EL * r:EL * (r + 1)],
                    io[f"kout{a}"][256 * r + 128 * m:256 * r + 128 * (m + 1), :])
        kT.append(kTa)
        vs = apool.tile([128, NFC * 260], F16, name=f"v_store_{a}", tag=f"v_store_{a}")
        for t in range(NFC):
            nc.sync.dma_start(vs[:, 260 * t:260 * (t + 1)],
                              io[f"vout{a}"][128 * t:128 * (t + 1), :])
        v_store.append(vs)

    # ---------------- gate path ----------------
    hg_ps = pstile()
    hg_psv = hg_ps[0:cfg.DG, 0:EL]
    nc.tensor.matmul(hg_psv, wg1[0][:], xTl[0][:], start=True, stop=False)
    nc.tensor.matmul(hg_psv, wg1[1][:], xTl[1][:], start=False, stop=True)

    def gelu2(z_ps, bias_ap, out_f16, P, N):
        """out = z*(1+tanh(c*(z+GELU_C*z^3))) = 2*gelu(z), z = z_ps + bias."""
        z = tmp.tile([128, 512], F32, name="gelu_z", tag="gelu_z")
        nc.vector.tensor_scalar(z[0:P, 0:N], z_ps, bias_ap, None, op0=ADD)
        u = tmp.tile([128, 512], F32, name="gelu_u", tag="gelu_t")
        nc.vector.tensor_tensor(u[0:P, 0:N], z[0:P, 0:N], z[0:P, 0:N], op=MULT)
        w = tmp.tile([128, 512], F32, name="gelu_w", tag="gelu_t")
        nc.vector.tensor_scalar(w[0:P, 0:N], u[0:P, 0:N], GELU_C, 1.0, op0=MULT, op1=ADD)
        t2 = tmp.tile([128, 512], F32, name="gelu_t2", tag="gelu_t")
        nc.vector.tensor_tensor(t2[0:P, 0:N], z[0:P, 0:N], w[0:P, 0:N], op=MULT)
        th = tmp.tile([128, 512], F32, name="gelu_th", tag="gelu_t")
        nc.scalar.activation(th[0:P, 0:N], t2[0:P, 0:N], AF.Tanh, scale=SQRT_2_OVER_PI)
        a2 = tmp.tile([128, 512], F32, name="gelu_a2", tag="gelu_t")
        nc.vector.tensor_scalar(a2[0:P, 0:N], th[0:P, 0:N], 1.0, None, op0=ADD)
        nc.vector.tensor_tensor(out_f16, z[0:P, 0:N], a2[0:P, 0:N], op=MULT)

    def stile(nm, shape, dtype):
        return small.tile(shape, dtype, name=nm, tag=nm)

    hg = stile("hg", [cfg.DG, EL], F16)  # = 2*gelu(gate pre-act)
    gelu2(hg_psv, bg1[:], hg[:], cfg.DG, EL)

    al_ps = pstile()
    nc.tensor.matmul(al_ps[0:1, 0:EL], wg2h[:], hg[:], start=True, stop=True)
    tha = stile("tha", [1, EL], F32)
    # sigmoid(l) = 0.5 + 0.5*tanh(0.5*l); wg2h has 0.25*Wg2 folded in
    nc.scalar.activation(tha[:], al_ps[0:1, 0:EL], AF.Tanh, scale=1.0, bias=bg2h[:])
    alpha = stile("alpha", [1, EL], F32)
    nc.vector.tensor_scalar(alpha[:], tha[:], 0.5, 0.5, op0=MULT, op1=ADD)
    ac = stile("ac", [1, EL], F32)
    nc.vector.tensor_scalar(ac[:], alpha[:], -1.0, 1.0, op0=MULT, op1=ADD)
    alpha16 = stile("alpha16", [1, EL], F16)
    nc.vector.tensor_copy(alpha16[:], alpha[:])
    ac16 = stile("ac16", [1, EL], F16)
    nc.vector.tensor_copy(ac16[:], ac[:])
    apart = stile("apart", [1, 1], F32)
    nc.vector.reduce_sum(apart[:], alpha[:], axis=mybir.AxisListType.X)
    nc.gpsimd.dma_start(io["alpha_part"][:], apart[:])

    ab = stile("ab", [128, EL], F32)
    ab_ps = pstile()
    nc.tensor.matmul(ab_ps[:, 0:EL], ones_row[:], alpha16[:], start=True, stop=True)
    nc.vector.tensor_copy(ab[:], ab_ps[:, 0:EL])
    acb = stile("acb", [128, EL], F32)
    acb_ps = pstile()
    nc.tensor.matmul(acb_ps[:, 0:EL], ones_row[:], ac16[:], start=True, stop=True)
    nc.vector.tensor_copy(acb[:], acb_ps[:, 0:EL])

    # ---------------- attention head loops ----------------
    t1s = []
    for a in range(2):
        outT = [apool.tile([128, EL], F16, name=f"outT{m}_{a}", tag=f"outT{m}_{a}")
                for m in range(2)]
        rb = [apool.tile([128, EL], F32, name=f"rb{m}_{a}", tag=f"rb{m}_{a}")
              for m in range(2)]
        for pair in range(2):
            avacc = [ps512.tile([128, 512], F32, name=f"avacc{hh}", tag="ps")
                     for hh in range(2)]
            for s in range(cfg.n_stripes):
                w_s = min(CS, NFC - CS * s)
                Ns = EL * w_s
                mt = stream.tile([128, EL * CS], F16, name="mt", tag="mask")
                for j in range(w_s):
                    nc.sync.dma_start(mt[:, EL * j:EL * (j + 1)],
                                      io[f"mask{a}"][CS * s + j])
                for hh in range(2):
                    h = 2 * pair + hh
                    mi, po = h // 2, 64 * (h % 2)
                    sc = scp.tile([128, EL * CS], F32, name="sc", tag="sc")
                    for j in range(w_s):
                        t = CS * s + j
                        nc.tensor.matmul(
                            sc[0:128, EL * j:EL * j + EL],
                            kT[a][mi][po:po + 64, 128 * t:128 * (t + 1)],
                            qT[a][mi][po:po + 64, :],
                            start=True, stop=True)
                    pm = stream.tile([128, EL * CS], F16, name="pm", tag="pm")
                    nc.scalar.activation(pm[:, 0:Ns], sc[:, 0:Ns], AF.Exp, scale=0.125)
                    pmm = stream.tile([128, EL * CS], F16, name="pmm", tag="pmm")
                    eng = nc.gpsimd if (s % 4 == 1) else nc.vector
                    eng.tensor_tensor(pmm[:, 0:Ns], pm[:, 0:Ns], mt[:, 0:Ns], op=MULT)
                    for j in range(w_s):
                        t = CS * s + j
                        nc.tensor.matmul(
                            avacc[hh][0:65, 0:EL],
                            v_store[a][:, 260 * t + 65 * h:260 * t + 65 * h + 65],
                            pmm[:, EL * j:EL * j + EL],
                            start=(t == 0), stop=(t == NFC - 1))
            # normalize: out[d, e] = avacc[d, e] * (1 / avacc[64, e])
            mi = pair
            rb_ps = scp.tile([128, EL * CS], F32, name="rb_ps", tag="sc")
            for hh in range(2):
                rcp = stile(f"rcp{a}_{pair}_{hh}", [1, EL], F32)
                nc.vector.reciprocal(rcp[:], avacc[hh][64:65, 0:EL])
                rcph = stile(f"rcph{a}_{pair}_{hh}", [1, EL], F16)
                nc.vector.tensor_copy(rcph[:], rcp[:])
                nc.tensor.matmul(rb_ps[64 * hh:64 * hh + 64, 0:EL],
                                 ones_row[0:1, 0:64], rcph[:],
                                 start=True, stop=True)
            nc.vector.tensor_copy(rb[mi][:], rb_ps[:, 0:EL])
            for hh in range(2):
                po = 64 * hh
                nc.vector.tensor_tensor(outT[mi][po:po + 64, :],
                                        avacc[hh][0:64, 0:EL],
                                        rb[mi][po:po + 64, :], op=MULT)

        # ---- P = Wo^T @ outT, pre-multiplied by its blend coefficient ----
        cb = acb if a == 0 else ab
        t1m = []
        for m in range(2):
            ps = pstile()
            psv = ps[:, 0:EL]
            nc.tensor.matmul(psv, wo[a][0][:, 128 * m:128 * (m + 1)], outT[0][:],
                             start=True, stop=False)
            nc.tensor.matmul(psv, wo[a][1][:, 128 * m:128 * (m + 1)], outT[1][:],
                             start=False, stop=True)
            t1 = tmp.tile([128, 512], F32, name=f"t1_{a}_{m}", tag=f"t1_{a}_{m}")
            nc.vector.tensor_tensor(t1[:, 0:EL], psv, cb[:], op=MULT)
            t1m.append(t1)
        t1s.append(t1m)

    # ---------------- blend + classifier ----------------
    blended = [stile(f"blend{m}", [128, EL], F16) for m in range(2)]
    for m in range(2):
        bb = pstile()
        bbv = bb[:, 0:EL]
        nc.tensor.matmul(bbv, bo[0][0:1, 128 * m:128 * (m + 1)], ac16[:], start=True, stop=False)
        nc.tensor.matmul(bbv, bo[1][0:1, 128 * m:128 * (m + 1)], alpha16[:], start=False, stop=True)
        t3 = tmp.tile([128, 512], F32, name="bl_t3", tag="bl_t3")
        nc.vector.tensor_tensor(t3[:, 0:EL], t1s[0][m][:, 0:EL], t1s[1][m][:, 0:EL], op=ADD)
        nc.vector.tensor_tensor(blended[m][:], t3[:, 0:EL], bbv, op=ADD)

    chT = [stile(f"ch{m}", [128, EL], F16) for m in range(2)]
    for m in range(2):
        ps = pstile()
        psv = ps[:, 0:EL]
        nc.tensor.matmul(psv, wc1[0][:, 128 * m:128 * (m + 1)], blended[0][:], start=True, stop=False)
        nc.tensor.matmul(psv, wc1[1][:, 128 * m:128 * (m + 1)], blended[1][:], start=False, stop=True)
        gelu2(psv, bc1[m][:], chT[m][:], 128, EL)
    # logitsT = Wc2'^T @ chT + bc2, with the 0.5 of gelu2 folded into Wc2'
    lp = pstile()
    lpv = lp[0:cfg.NCLS, 0:EL]
    nc.tensor.matmul(lpv, wc2[0][:], chT[0][:], start=True, stop=False)
    nc.tensor.matmul(lpv, wc2[1][:], chT[1][:], start=False, stop=True)
    logT = stile("logT", [cfg.NCLS, EL], F32)
    nc.vector.tensor_scalar(logT[:], lpv, bc2[:], None, op0=ADD)
    nc.sync.dma_start(io["logitsT"][:], logT[:])


def build_nc(cfg, reps=1, dbg=False):
    nc = bacc.Bacc("TRN2", target_bir_lowering=False, debug=False,
                   num_devices=cfg.NC)
    io = {}

    def din(name, shape, dtype):
        io[name] = nc.dram_tensor(name, shape, dtype, kind="ExternalInput").ap()

    E = cfg.E
    din("xTl", [256, cfg.EL], F16)
    for a in range(2):
        din(f"wqkv{a}", [256, 768], F16)
        din(f"bqkv{a}", [768, 1], F32)
        din(f"wv_aug{a}", [256, 260], F16)
        din(f"bv_aug{a}", [1, 260], F16)
        din(f"wo{a}", [256, 256], F16)
        din(f"bo{a}", [1, 256], F16)
        din(f"mask{a}", [cfg.NFC, 128, cfg.EL], F16)
        io[f"kin{a}"] = nc.dram_tensor(f"kin{a}", [256, cfg.EL], F16).ap()
        io[f"kout{a}"] = nc.dram_tensor(f"kout{a}", [256 * cfg.NC, cfg.EL], F16,
                                        addr_space="Shared").ap()
        io[f"vin{a}"] = nc.dram_tensor(f"vin{a}", [cfg.EL, 260], F16).ap()
        io[f"vout{a}"] = nc.dram_tensor(f"vout{a}", [E, 260], F16,
                                        addr_space="Shared").ap()
    din("wg1", [256, cfg.DG], F16)
    din("bg1", [cfg.DG, 1], F32)
    din("wg2h", [cfg.DG, 1], F16)
    din("bg2h", [1, 1], F32)
    din("wc1", [256, 256], F16)
    din("bc1", [256, 1], F32)
    din("wc2", [256, cfg.NCLS], F16)
    din("bc2", [cfg.NCLS, 1], F32)
    io["logitsT"] = nc.dram_tensor("logitsT", [cfg.NCLS, cfg.EL], F32,
                                   kind="ExternalOutput").ap()
    io["alpha_part"] = nc.dram_tensor("alpha_part", [1, 1], F32,
                                      kind="ExternalOutput").ap()

    from contextlib import ExitStack
    with tile.TileContext(nc) as tc:
        for rep in range(reps):
            with ExitStack() as ctx:
                _emit_core_program(nc, tc, ctx, io, cfg, rep, dbg=dbg)
    nc.compile()
    return nc


def host_prep(cfg, inputs):
    """Build the per-core input maps from the full problem inputs."""
    E, EL, NFC = cfg.E, cfg.EL, cfg.NFC
    f16 = np.float16
    x = np.asarray(inputs["edge_features"], np.float32)
    xT = np.ascontiguousarray(x.T).astype(f16)
    shared = {}
    for a, (wq, bqv, wo_, bo_) in enumerate([
        (inputs["Wqkv1"], inputs["bqkv1"], inputs["Wo1"], inputs["bo1"]),
        (inputs["Wqkv2"], inputs["bqkv2"], inputs["Wo2"], inputs["bo2"]),
    ]):
        wq = np.asarray(wq, np.float32)
        bqv = np.asarray(bqv, np.float32)
        shared[f"wqkv{a}"] = wq.astype(f16)
        shared[f"bqkv{a}"] = np.ascontiguousarray(bqv.reshape(768, 1))
        wv_aug = np.zeros((256, 260), np.float32)
        bv_aug = np.zeros((1, 260), np.float32)
        for h in range(4):
            wv_aug[:, 65 * h:65 * h + 64] = wq[:, 512 + 64 * h:512 + 64 * (h + 1)]
            bv_aug[0, 65 * h:65 * h + 64] = bqv[512 + 64 * h:512 + 64 * (h + 1)]
            bv_aug[0, 65 * h + 64] = 1.0
        shared[f"wv_aug{a}"] = wv_aug.astype(f16)
        shared[f"bv_aug{a}"] = bv_aug.astype(f16)
        shared[f"wo{a}"] = np.asarray(wo_, np.float32).astype(f16)
        shared[f"bo{a}"] = np.asarray(bo_, np.float32).reshape(1, 256).astype(f16)
    shared["wg1"] = np.asarray(inputs["Wg1"], np.float32).astype(f16)
    shared["bg1"] = np.ascontiguousarray(
        np.asarray(inputs["bg1"], np.float32).reshape(cfg.DG, 1))
    shared["wg2h"] = (0.25 * np.asarray(inputs["Wg2"], np.float32)).astype(f16)
    shared["bg2h"] = (0.5 * np.asarray(inputs["bg2"], np.float32)).reshape(1, 1)
    shared["wc1"] = np.asarray(inputs["Wc1"], np.float32).astype(f16)
    shared["bc1"] = np.ascontiguousarray(
        np.asarray(inputs["bc1"], np.float32).reshape(256, 1))
    shared["wc2"] = (0.5 * np.asarray(inputs["Wc2"], np.float32)).astype(f16)
    shared["bc2"] = np.ascontiguousarray(
        np.asarray(inputs["bc2"], np.float32).reshape(cfg.NCLS, 1))

    adjT = [np.asarray(inputs["adj1"]).T, np.asarray(inputs["adj2"]).T]
    in_maps = []
    for c in range(cfg.NC):
        m = dict(shared)
        sl = slice(EL * c, EL * (c + 1))
        m["xTl"] = np.ascontiguousarray(xT[:, sl])
        for a in range(2):
            mt = adjT[a][:, sl].astype(f16)           # [E, EL]
            m[f"mask{a}"] = np.ascontiguousarray(mt.reshape(NFC, 128, EL))
        in_maps.append(m)
    return in_maps


_CACHE = {}


def _get_nc(cfg, reps=1):
    key = (cfg.E, cfg.NC, cfg.CS, reps)
    if key not in _CACHE:
        _CACHE[key] = build_nc(cfg, reps)
    return _CACHE[key]


def kernel(**inputs):
    cfg = Cfg()
    nc = _get_nc(cfg)
    in_maps = host_prep(cfg, inputs)
    res = run_bass_kernel_spmd(nc, in_maps, core_ids=list(range(cfg.NC)))
    logits = np.concatenate(
        [np.ascontiguousarray(res.results[c]["logitsT"].T) for c in range(cfg.NC)],
        axis=0)
    alpha_mean = np.float32(
        sum(float(res.results[c]["alpha_part"][0, 0]) for c in range(cfg.NC))
        / cfg.E)
    return logits.astype(np.float32), alpha_mean


# revision 31
# speedup vs baseline: 3.7092x; 3.7092x over previous
"""Trainium2 distributed kernel for nn_AdaptiveHopModel (gnn_message_passing).

Model: two masked edge-attention blocks over E=4096 edges (D=256, H=4), an
adaptive per-edge hop gate alpha = sigmoid(MLP(x)), blended output fed
through a small classifier; returns (logits [E,16], alpha.mean()).

Sharding: query-edge rows (E) split across 8 NeuronCores (512 rows/core),
weights replicated, K/V computed for all E on every core (no collectives).
Activations are kept in a transposed [feature, edge] layout so every x@W
matmul maps directly onto the TensorEngine.  The masked softmax streams:
PE scoresT -> ScalarE exp (PSUM->SBUF, fp16) -> DVE mask-multiply ->
PE attn@V with a ones-column in V producing row-sums for free;
normalization happens on the small [65, e] output.

Masks are pre-transposed/tiled on the host as fp16 {0,1}; alpha.mean() is
assembled on the host from per-core partials; logits come back transposed
[16, 512] per core and are transposed on the host.
"""

import sys

sys.path.insert(0, "/opt/trn_rl_repo")

import numpy as np

import concourse.bass as bass
import concourse.bacc as bacc
import concourse.tile as tile
import concourse.mybir as mybir
from concourse.bass_utils import run_bass_kernel_spmd

F16 = mybir.dt.float16
F32 = mybir.dt.float32
AF = mybir.ActivationFunctionType
MULT = mybir.AluOpType.mult
ADD = mybir.AluOpType.add

SQRT_2_OVER_PI = 0.7978845608028654
GELU_C = 0.044715


class Cfg:
    def __init__(self, E=4096, n_cores=8, CS=3):
        self.E = E                      # total edges
        self.D = 256                    # edge feature dim
        self.H = 4                      # heads
        self.DH = 64                    # head dim
        self.NC = n_cores
        self.EL = E // n_cores          # local query rows per core
        self.NFC = E // 128             # f-chunks of 128 key rows
        self.CS = CS                    # f-chunks per softmax stripe
        self.NCLS = 16
        self.DG = 128                   # gate hidden dim
        self.n_stripes = (self.NFC + CS - 1) // CS


def _emit_core_program(nc, tc, ctx, io, cfg, rep, dbg=False):
    """Emit the whole per-core program (one repetition)."""
    E, EL, NFC, CS = cfg.E, cfg.EL, cfg.NFC, cfg.CS
    R = f"r{rep}"

    konst = ctx.enter_context(tc.tile_pool(name=f"konst{R}", bufs=1))
    ps512 = ctx.enter_context(tc.tile_pool(name=f"ps512{R}", bufs=2, space="PSUM"))
    tmp = ctx.enter_context(tc.tile_pool(name=f"tmp{R}", bufs=2))
    small = ctx.enter_context(tc.tile_pool(name=f"small{R}", bufs=1))

    def ktile(nm, shape, dtype):
        return konst.tile(shape, dtype, name=nm, tag=nm)

    def load_const(nm, shape, dtype, eng=None):
        t = ktile(nm, shape, dtype)
        (eng or nc.gpsimd).dma_start(t[:], io[nm][:])
        return t

    def load_rows(nm, src, nrow=128, nsplit=1, eng=None):
        """Load a [n*nrow, C] dram tensor as n tiles of [nrow, C]."""
        eng = eng or nc.gpsimd
        n = io[src].shape[0] // nrow
        ts = []
        for d in range(n):
            t = ktile(f"{nm}{d}", [nrow, io[src].shape[1]], io[src].dtype)
            if nsplit == 1:
                eng.dma_start(t[:], io[src][nrow * d:nrow * (d + 1), :])
            else:
                step = io[src].shape[1] // nsplit
                for i in range(nsplit):
                    eng.dma_start(
                        t[:, step * i:step * (i + 1)],
                        io[src][nrow * d:nrow * (d + 1), step * i:step * (i + 1)])
            ts.append(t)
        return ts

    # ---------------- constant loads (packed) ----------------
    xTl = load_rows("xTl", "xTl", eng=nc.sync)
    xT = load_rows("xT", "xT", nsplit=4, eng=nc.sync)

    # wpack fp16 [128, WCOLS]: all [128, x] fp16 weight tiles as column blocks
    wpack = ktile("wpack", [128, io["wpack"].shape[1]], F16)
    nc.sync.dma_start(wpack[:], io["wpack"][:])
    # bpack f32 [128, x]: per-partition bias columns
    bpack = ktile("bpack", [128, io["bpack"].shape[1]], F32)
    nc.sync.dma_start(bpack[:], io["bpack"][:])
    # rpack fp16 [1, x]: row-vector constants
    rpack = ktile("rpack", [1, io["rpack"].shape[1]], F16)
    nc.sync.dma_start(rpack[:], io["rpack"][:])
    bg2h = ktile("bg2h", [1, 1], F32)
    nc.sync.dma_start(bg2h[:], io["bg2h"][:])

    wq_off = {}
    col = 0

    def wslice(width):
        nonlocal col
        s = wpack[:, col:col + width]
        col += width
        return s

    wqkv, wv_aug, wo = [], [], []
    for a in range(2):
        wqkv.append([wslice(768) for _ in range(2)])
        wv_aug.append([wslice(260) for _ in range(2)])
        wo.append([wslice(256) for _ in range(2)])
    wg1 = [wslice(cfg.DG) for _ in range(2)]
    wc1 = [wslice(256) for _ in range(2)]
    wc2 = [wslice(cfg.NCLS) for _ in range(2)]
    wg2h = wslice(1)

    bcol = 0

    def bslice(n=1):
        nonlocal bcol
        s = bpack[:, bcol:bcol + n]
        bcol += n
        return s

    bq, bk = [], []
    for a in range(2):
        bq.append([bslice() for _ in range(2)])
        bk.append([bslice() for _ in range(2)])
    bg1 = bslice()
    bc1 = [bslice() for _ in range(2)]
    bc2_full = bslice()
    bc2 = bc2_full[0:cfg.NCLS, :]

    rcol = 0

    def rslice(n):
        nonlocal rcol
        s = rpack[:, rcol:rcol + n]
        rcol += n
        return s

    bv_aug, bo = [], []
    for a in range(2):
        bv_aug.append(rslice(260))
        bo.append(rslice(256))

    ones_row = ktile("ones_row", [1, 128], F16)
    nc.vector.memset(ones_row[:], 1.0)
    io_dbg_av = ktile("io_dbg_av", [128, 512], F32) if dbg else None

    def pstile():
        return ps512.tile([128, 512], F32, name="ps", tag="ps")

    # ---------------- gate path ----------------
    hg_ps = pstile()
    hg_psv = hg_ps[0:cfg.DG, 0:EL]
    nc.tensor.matmul(hg_psv, wg1[0], xTl[0][:], start=True, stop=False)
    nc.tensor.matmul(hg_psv, wg1[1], xTl[1][:], start=False, stop=True)

    def gelu2(z_ps, bias_ap, out_f16, P, N):
        """out = z*(1+tanh(c*(z+GELU_C*z^3))) = 2*gelu(z), z = z_ps + bias."""
        z = tmp.tile([128, 512], F32, name="gelu_z", tag="gelu_z")
        nc.vector.tensor_scalar(z[0:P, 0:N], z_ps, bias_ap, None, op0=ADD)
        u = tmp.tile([128, 512], F32, name="gelu_u", tag="gelu_t")
        nc.vector.tensor_tensor(u[0:P, 0:N], z[0:P, 0:N], z[0:P, 0:N], op=MULT)
        w = tmp.tile([128, 512], F32, name="gelu_w", tag="gelu_t")
        nc.vector.tensor_scalar(w[0:P, 0:N], u[0:P, 0:N], GELU_C, 1.0, op0=MULT, op1=ADD)
        t2 = tmp.tile([128, 512], F32, name="gelu_t2", tag="gelu_t")
        nc.vector.tensor_tensor(t2[0:P, 0:N], z[0:P, 0:N], w[0:P, 0:N], op=MULT)
        th = tmp.tile([128, 512], F32, name="gelu_th", tag="gelu_t")
        nc.scalar.activation(th[0:P, 0:N], t2[0:P, 0:N], AF.Tanh, scale=SQRT_2_OVER_PI)
        a2 = tmp.tile([128, 512], F32, name="gelu_a2", tag="gelu_t")
        nc.vector.tensor_scalar(a2[0:P, 0:N], th[0:P, 0:N], 1.0, None, op0=ADD)
        nc.vector.tensor_tensor(out_f16, z[0:P, 0:N], a2[0:P, 0:N], op=MULT)

    def stile(nm, shape, dtype):
        return small.tile(shape, dtype, name=nm, tag=nm)

    hg = stile("hg", [cfg.DG, EL], F16)  # = 2*gelu(gate pre-act)
    gelu2(hg_psv, bg1, hg[:], cfg.DG, EL)

    al_ps = pstile()
    nc.tensor.matmul(al_ps[0:1, 0:EL], wg2h, hg[:], start=True, stop=True)
    tha = stile("tha", [1, EL], F32)
    # sigmoid(l) = 0.5 + 0.5*tanh(0.5*l); wg2h has 0.25*Wg2 folded in
    nc.scalar.activation(tha[:], al_ps[0:1, 0:EL], AF.Tanh, scale=1.0, bias=bg2h[:])
    alpha = stile("alpha", [1, EL], F32)
    nc.vector.tensor_scalar(alpha[:], tha[:], 0.5, 0.5, op0=MULT, op1=ADD)
    ac = stile("ac", [1, EL], F32)
    nc.vector.tensor_scalar(ac[:], alpha[:], -1.0, 1.0, op0=MULT, op1=ADD)
    alpha16 = stile("alpha16", [1, EL], F16)
    nc.vector.tensor_copy(alpha16[:], alpha[:])
    ac16 = stile("ac16", [1, EL], F16)
    nc.vector.tensor_copy(ac16[:], ac[:])
    apart = stile("apart", [1, 1], F32)
    nc.vector.reduce_sum(apart[:], alpha[:], axis=mybir.AxisListType.X)
    nc.gpsimd.dma_start(io["alpha_part"][:], apart[:])

    ab = stile("ab", [128, EL], F32)
    ab_ps = pstile()
    nc.tensor.matmul(ab_ps[:, 0:EL], ones_row[:], alpha16[:], start=True, stop=True)
    nc.vector.tensor_copy(ab[:], ab_ps[:, 0:EL])
    acb = stile("acb", [128, EL], F32)
    acb_ps = pstile()
    nc.tensor.matmul(acb_ps[:, 0:EL], ones_row[:], ac16[:], start=True, stop=True)
    nc.vector.tensor_copy(acb[:], acb_ps[:, 0:EL])
    if dbg:
        nc.sync.dma_start(io["d_ab"][:], ab[:])
        nc.sync.dma_start(io["d_alpha"][:], alpha[:])
        nc.sync.dma_start(io["d_hg"][:], hg[:])
        nc.sync.dma_start(io["d_tha"][:], tha[:])
        hgpcp = tmp.tile([128, 512], F32, name="hgpcp", tag="hgpcp")
        nc.vector.tensor_copy(hgpcp[:, 0:EL], hg_psv)
        nc.sync.dma_start(io["d_hgps"][:], hgpcp[:, 0:EL])

    # ---------------- attention blocks ----------------
    P_sb = []  # per-attn Wo^T @ outT_norm in SBUF f32, 2 chunks of [128, EL]
    for a in range(2):
        with tc.tile_pool(name=f"attn{a}{R}", bufs=1) as apool, \
             tc.tile_pool(name=f"sc{a}{R}", bufs=2, space="PSUM") as scp, \
             tc.tile_pool(name=f"stream{a}{R}", bufs=4) as stream:

            def atile(nm, shape, dtype):
                return apool.tile(shape, dtype, name=f"{nm}_{a}", tag=nm)

            # ---- qT [256, EL] fp16 ----
            qT = [atile(f"qT{m}", [128, EL], F16) for m in range(2)]
            for m in range(2):
                ps = pstile()
                psv = ps[:, 0:EL]
                nc.tensor.matmul(psv, wqkv[a][0][:, 128 * m:128 * (m + 1)],
                                 xTl[0][:], start=True, stop=False)
                nc.tensor.matmul(psv, wqkv[a][1][:, 128 * m:128 * (m + 1)],
                                 xTl[1][:], start=False, stop=True)
                nc.vector.tensor_scalar(qT[m][:], psv, bq[a][m], None, op0=ADD)
                if dbg and a == (dbg - 1) and m == 0:
                    qpscp = tmp.tile([128, 512], F32, name="qpscp", tag="qpscp")
                    nc.vector.tensor_copy(qpscp[:, 0:EL], psv)
                    nc.sync.dma_start(io["d_qps"][:], qpscp[:, 0:EL])
            # ---- kT [256, E] fp16 ----
            kT = [atile(f"kT{m}", [128, E], F16) for m in range(2)]
            for m in range(2):
                for nn in range(E // 512):
                    ps = pstile()
                    sl = slice(512 * nn, 512 * (nn + 1))
                    nc.tensor.matmul(ps[:], wqkv[a][0][:, 256 + 128 * m:256 + 128 * (m + 1)],
                                     xT[0][:, sl], start=True, stop=False)
                    nc.tensor.matmul(ps[:], wqkv[a][1][:, 256 + 128 * m:256 + 128 * (m + 1)],
                                     xT[1][:, sl], start=False, stop=True)
                    nc.vector.tensor_scalar(kT[m][:, sl], ps[:], bk[a][m], None, op0=ADD)
            # ---- v_store [128, NFC*260] fp16 (per f-tile: 4 heads x (64 v + ones)) ----
            v_store = atile("v_store", [128, NFC * 260], F16)
            for t in range(NFC):
                ps = pstile()
                psv = ps[:, 0:260]
                fsl = slice(128 * t, 128 * (t + 1))
                nc.tensor.matmul(psv, xT[0][:, fsl], wv_aug[a][0], start=True, stop=False)
                nc.tensor.matmul(psv, xT[1][:, fsl], wv_aug[a][1], start=False, stop=False)
                nc.tensor.matmul(psv, ones_row[:], bv_aug[a], start=False, stop=True)
                nc.vector.tensor_copy(v_store[:, 260 * t:260 * (t + 1)], psv)

            # ---- heads: stripe loop, head pairs share the mask DMA ----
            outT = [atile(f"outT{m}", [128, EL], F16) for m in range(2)]
            rb = [atile(f"rb{m}", [128, EL], F32) for m in range(2)]
            for pair in range(2):
                avacc = [ps512.tile([128, 512], F32, name=f"avacc{hh}", tag="ps")
                         for hh in range(2)]
                for s in range(cfg.n_stripes):
                    w_s = min(CS, NFC - CS * s)
                    Ns = EL * w_s
                    mt = stream.tile([128, EL * CS], F16, name="mt", tag="mask")
                    for j in range(w_s):
                        nc.sync.dma_start(mt[:, EL * j:EL * (j + 1)],
                                          io[f"mask{a}"][CS * s + j])
                    for hh in range(2):
                        h = 2 * pair + hh
                        mi, po = h // 2, 64 * (h % 2)
                        sc = scp.tile([128, EL * CS], F32, name="sc", tag="sc")
                        for j in range(w_s):
                            t = CS * s + j
                            nc.tensor.matmul(
                                sc[0:128, EL * j:EL * j + EL],
                                kT[mi][po:po + 64, 128 * t:128 * (t + 1)],
                                qT[mi][po:po + 64, :],
                                start=True, stop=True)
                        pm = stream.tile([128, EL * CS], F16, name="pm", tag="pm", bufs=6)
                        nc.scalar.activation(pm[:, 0:Ns], sc[:, 0:Ns], AF.Exp, scale=0.125)
                        pmm = stream.tile([128, EL * CS], F16, name="pmm", tag="pmm", bufs=6)
                        nc.vector.tensor_tensor(pmm[:, 0:Ns], pm[:, 0:Ns], mt[:, 0:Ns], op=MULT)
                        if dbg and a == (dbg - 1) and pair == 0 and s == 0 and hh == 0:
                            nc.sync.dma_start(io["d_pmm"][:], pmm[:, 0:EL])
                            nc.sync.dma_start(io["d_pm"][:], pm[:, 0:EL])
                            nc.sync.dma_start(io["d_mt"][:], mt[:, 0:EL])
                        for j in range(w_s):
                            t = CS * s + j
                            nc.tensor.matmul(
                                avacc[hh][0:65, 0:EL],
                                v_store[:, 260 * t + 65 * h:260 * t + 65 * h + 65],
                                pmm[:, EL * j:EL * j + EL],
                                start=(t == 0), stop=(t == NFC - 1))
                # normalize: out[d, e] = avacc[d, e] * (1 / avacc[64, e])
                mi = pair
                rb_ps = scp.tile([128, EL * CS], F32, name="rb_ps", tag="sc")
                for hh in range(2):
                    rcp = stile(f"rcp{a}_{pair}_{hh}", [1, EL], F32)
                    nc.vector.reciprocal(rcp[:], avacc[hh][64:65, 0:EL])
                    rcph = stile(f"rcph{a}_{pair}_{hh}", [1, EL], F16)
                    nc.vector.tensor_copy(rcph[:], rcp[:])
                    nc.tensor.matmul(rb_ps[64 * hh:64 * hh + 64, 0:EL],
                                     ones_row[0:1, 0:64], rcph[:],
                                     start=True, stop=True)
                nc.vector.tensor_copy(rb[mi][:], rb_ps[:, 0:EL])
                for hh in range(2):
                    po = 64 * hh
                    nc.vector.tensor_tensor(outT[mi][po:po + 64, :],
                                            avacc[hh][0:64, 0:EL],
                                            rb[mi][po:po + 64, :], op=MULT)
                    if dbg and a == (dbg - 1) and h == 0:
                        nc.vector.tensor_copy(io_dbg_av[0:65, 0:EL], avacc[hh][0:65, 0:EL])
                        nc.sync.dma_start(io["d_avacc"][:], io_dbg_av[0:65, 0:EL])
                        nc.sync.dma_start(io["d_rb"][:], rb[0][:])

            # ---- P = Wo^T @ outT  (f32 in SBUF; bias folded at blend) ----
            Pm = []
            cb = acb if a == 0 else ab
            for m in range(2):
                ps = pstile()
                psv = ps[:, 0:EL]
                nc.tensor.matmul(psv, wo[a][0][:, 128 * m:128 * (m + 1)], outT[0][:],
                                 start=True, stop=False)
                nc.tensor.matmul(psv, wo[a][1][:, 128 * m:128 * (m + 1)], outT[1][:],
                                 start=False, stop=True)
                p_sb = tmp.tile([128, 512], F32, name=f"P{a}_{m}", tag=f"P{a}_{m}")
                nc.vector.tensor_tensor(p_sb[:, 0:EL], psv, cb[:], op=MULT)
                Pm.append(p_sb)
            if dbg and a == (dbg - 1):
                nc.sync.dma_start(io["d_outT"][:], outT[0][:])
                nc.sync.dma_start(io["d_P"][:], Pm[0][:, 0:EL])
                nc.sync.dma_start(io["d_qT"][:], qT[0][:])
                nc.sync.dma_start(io["d_kT"][:], kT[0][:, 0:EL])
                nc.sync.dma_start(io["d_v"][:], v_store[:, 0:260])
            P_sb.append(Pm)

    # ---------------- blend + classifier ----------------
    blended = [stile(f"blend{m}", [128, EL], F16) for m in range(2)]
    for m in range(2):
        bb = pstile()
        bbv = bb[:, 0:EL]
        nc.tensor.matmul(bbv, bo[0][0:1, 128 * m:128 * (m + 1)], ac16[:], start=True, stop=False)
        nc.tensor.matmul(bbv, bo[1][0:1, 128 * m:128 * (m + 1)], alpha16[:], start=False, stop=True)
        t3 = tmp.tile([128, 512], F32, name="bl_t3", tag="bl_t3")
        nc.vector.tensor_tensor(t3[:, 0:EL], P_sb[0][m][:, 0:EL], P_sb[1][m][:, 0:EL], op=ADD)
        nc.vector.tensor_tensor(blended[m][:], t3[:, 0:EL], bbv, op=ADD)
        if dbg and m == 0:
            bbcp = tmp.tile([128, 512], F32, name="bbcp", tag="bbcp")
            nc.vector.tensor_copy(bbcp[:, 0:EL], bbv)
            nc.sync.dma_start(io["d_bb"][:], bbcp[:, 0:EL])
            nc.sync.dma_start(io["d_t3"][:], t3[:, 0:EL])
            nc.sync.dma_start(io["d_blended"][:], blended[0][:])
            nc.sync.dma_start(io["d_t1"][:], t1[:, 0:EL])
            nc.sync.dma_start(io["d_t2"][:], t2[:, 0:EL])
            nc.sync.dma_start(io["d_ab2"][:], ab[:])
            nc.sync.dma_start(io["d_acb2"][:], acb[:])
            nc.sync.dma_start(io["d_P00"][:], P_sb[0][0][:, 0:EL])
            nc.sync.dma_start(io["d_P10"][:], P_sb[1][0][:, 0:EL])

    chT = [stile(f"ch{m}", [128, EL], F16) for m in range(2)]
    for m in range(2):
        ps = pstile()
        psv = ps[:, 0:EL]
        nc.tensor.matmul(psv, wc1[0][:, 128 * m:128 * (m + 1)], blended[0][:], start=True, stop=False)
        nc.tensor.matmul(psv, wc1[1][:, 128 * m:128 * (m + 1)], blended[1][:], start=False, stop=True)
        gelu2(psv, bc1[m], chT[m][:], 128, EL)
        if dbg and m == 0:
            zcp = tmp.tile([128, 512], F32, name="zcp", tag="zcp")
            nc.vector.tensor_copy(zcp[:, 0:EL], psv)
            nc.sync.dma_start(io["d_zc"][:], zcp[:, 0:EL])
            nc.sync.dma_start(io["d_ch"][:], chT[0][:])
    # logitsT = Wc2'^T @ chT + bc2, with the 0.5 of gelu2 folded into Wc2'
    lp = pstile()
    lpv = lp[0:cfg.NCLS, 0:EL]
    nc.tensor.matmul(lpv, wc2[0], chT[0][:], start=True, stop=False)
    nc.tensor.matmul(lpv, wc2[1], chT[1][:], start=False, stop=True)
    logT = stile("logT", [cfg.NCLS, EL], F32)
    nc.vector.tensor_scalar(logT[:], lpv, bc2, None, op0=ADD)
    nc.sync.dma_start(io["logitsT"][:], logT[:])


def build_nc(cfg, reps=1, dbg=False):
    nc = bacc.Bacc("TRN2", target_bir_lowering=False, debug=False,
                   num_devices=cfg.NC)
    io = {}

    def din(name, shape, dtype):
        io[name] = nc.dram_tensor(name, shape, dtype, kind="ExternalInput").ap()

    E = cfg.E
    din("xT", [256, E], F16)
    din("xTl", [256, cfg.EL], F16)
    for a in range(2):
        din(f"mask{a}", [cfg.NFC, 128, cfg.EL], F16)
    WCOLS = 2 * (2 * 768 + 2 * 260 + 2 * 256) + 2 * cfg.DG + 2 * 256 + 2 * cfg.NCLS + 1
    din("wpack", [128, WCOLS], F16)
    din("bpack", [128, 12], F32)
    din("rpack", [1, 2 * (260 + 256)], F16)
    din("bg2h", [1, 1], F32)
    io["logitsT"] = nc.dram_tensor("logitsT", [cfg.NCLS, cfg.EL], F32,
                                   kind="ExternalOutput").ap()
    io["alpha_part"] = nc.dram_tensor("alpha_part", [1, 1], F32,
                                      kind="ExternalOutput").ap()
    if dbg:
        for nm, shape, dt in [
            ("d_ab", [128, cfg.EL], F32), ("d_alpha", [1, cfg.EL], F32),
            ("d_pmm", [128, cfg.EL], F16), ("d_pm", [128, cfg.EL], F16),
            ("d_mt", [128, cfg.EL], F16), ("d_avacc", [65, cfg.EL], F32),
            ("d_rb", [128, cfg.EL], F32), ("d_outT", [128, cfg.EL], F16),
            ("d_P", [128, cfg.EL], F32), ("d_qT", [128, cfg.EL], F16),
            ("d_kT", [128, cfg.EL], F16), ("d_v", [128, 260], F16),
            ("d_bb", [128, cfg.EL], F32), ("d_t3", [128, cfg.EL], F32),
            ("d_blended", [128, cfg.EL], F16), ("d_zc", [128, cfg.EL], F32),
            ("d_ch", [128, cfg.EL], F16),
            ("d_t1", [128, cfg.EL], F32), ("d_t2", [128, cfg.EL], F32),
            ("d_ab2", [128, cfg.EL], F32), ("d_acb2", [128, cfg.EL], F32),
            ("d_P00", [128, cfg.EL], F32), ("d_P10", [128, cfg.EL], F32),
            ("d_qps", [128, cfg.EL], F32),
            ("d_hg", [128, cfg.EL], F16), ("d_tha", [1, cfg.EL], F32),
            ("d_hgps", [128, cfg.EL], F32),
        ]:
            io[nm] = nc.dram_tensor(nm, shape, dt, kind="ExternalOutput").ap()

    from contextlib import ExitStack
    with tile.TileContext(nc) as tc:
        for rep in range(reps):
            with ExitStack() as ctx:
                _emit_core_program(nc, tc, ctx, io, cfg, rep, dbg=dbg)
    nc.compile()
    return nc


def host_prep(cfg, inputs):
    """Build the per-core input maps from the full problem inputs."""
    E, EL, NFC = cfg.E, cfg.EL, cfg.NFC
    f16 = np.float16
    x = np.asarray(inputs["edge_features"], np.float32)
    xT = np.ascontiguousarray(x.T).astype(f16)
    shared = {"xT": xT}
    wcols, bcols, rcols = [], [], []
    for a, (wq, bqv, wo_, bo_) in enumerate([
        (inputs["Wqkv1"], inputs["bqkv1"], inputs["Wo1"], inputs["bo1"]),
        (inputs["Wqkv2"], inputs["bqkv2"], inputs["Wo2"], inputs["bo2"]),
    ]):
        wq = np.asarray(wq, np.float32)
        bqv = np.asarray(bqv, np.float32)
        wo_ = np.asarray(wo_, np.float32)
        bo_ = np.asarray(bo_, np.float32)
        wv_aug = np.zeros((256, 260), np.float32)
        bv_aug = np.zeros((1, 260), np.float32)
        for h in range(4):
            wv_aug[:, 65 * h:65 * h + 64] = wq[:, 512 + 64 * h:512 + 64 * (h + 1)]
            bv_aug[0, 65 * h:65 * h + 64] = bqv[512 + 64 * h:512 + 64 * (h + 1)]
            bv_aug[0, 65 * h + 64] = 1.0
        wcols += [wq[0:128], wq[128:256], wv_aug[0:128], wv_aug[128:256],
                  wo_[0:128], wo_[128:256]]
        bcols += [bqv[0:128, None], bqv[128:256, None],
                  bqv[256:384, None], bqv[384:512, None]]
        rcols += [bv_aug, bo_.reshape(1, 256)]
    wg1 = np.asarray(inputs["Wg1"], np.float32)
    wc1 = np.asarray(inputs["Wc1"], np.float32)
    wc2h = 0.5 * np.asarray(inputs["Wc2"], np.float32)
    wg2h = 0.25 * np.asarray(inputs["Wg2"], np.float32)
    wcols += [wg1[0:128], wg1[128:256], wc1[0:128], wc1[128:256],
              wc2h[0:128], wc2h[128:256], wg2h[0:128]]
    bc2c = np.zeros((128, 1), np.float32)
    bc2c[0:16, 0] = np.asarray(inputs["bc2"], np.float32)
    bcols += [np.asarray(inputs["bg1"], np.float32).reshape(128, 1),
              np.asarray(inputs["bc1"], np.float32)[0:128, None],
              np.asarray(inputs["bc1"], np.float32)[128:256, None],
              bc2c]
    shared["wpack"] = np.ascontiguousarray(np.concatenate(wcols, axis=1)).astype(f16)
    shared["bpack"] = np.ascontiguousarray(np.concatenate(bcols, axis=1)).astype(np.float32)
    shared["rpack"] = np.ascontiguousarray(np.concatenate(rcols, axis=1)).astype(f16)
    shared["bg2h"] = (0.5 * np.asarray(inputs["bg2"], np.float32)).reshape(1, 1)

    adjT = [np.asarray(inputs["adj1"]).T, np.asarray(inputs["adj2"]).T]
    in_maps = []
    for c in range(cfg.NC):
        m = dict(shared)
        sl = slice(EL * c, EL * (c + 1))
        m["xTl"] = np.ascontiguousarray(shared["xT"][:, sl])
        for a in range(2):
            mt = adjT[a][:, sl].astype(f16)           # [E, EL]
            m[f"mask{a}"] = np.ascontiguousarray(mt.reshape(NFC, 128, EL))
        in_maps.append(m)
    return in_maps


_CACHE = {}


def _get_nc(cfg, reps=1):
    key = (cfg.E, cfg.NC, cfg.CS, reps)
    if key not in _CACHE:
        _CACHE[key] = build_nc(cfg, reps)
    return _CACHE[key]


def kernel(**inputs):
    cfg = Cfg()
    nc = _get_nc(cfg)
    in_maps = host_prep(cfg, inputs)
    res = run_bass_kernel_spmd(nc, in_maps, core_ids=list(range(cfg.NC)))
    logits = np.concatenate(
        [np.ascontiguousarray(res.results[c]["logitsT"].T) for c in range(cfg.NC)],
        axis=0)
    alpha_mean = np.float32(
        sum(float(res.results[c]["alpha_part"][0, 0]) for c in range(cfg.NC))
        / cfg.E)
    return logits.astype(np.float32), alpha_mean


# revision 38
# speedup vs baseline: 4.4104x; 1.1890x over previous
"""Trainium2 distributed kernel for nn_AdaptiveHopModel (gnn_message_passing).

Model: two masked edge-attention blocks over E=4096 edges (D=256, H=4), an
adaptive per-edge hop gate alpha = sigmoid(MLP(x)), blended output fed
through a small classifier; returns (logits [E,16], alpha.mean()).

Sharding: query-edge rows (E) split across 8 NeuronCores (512 rows/core),
weights replicated, K/V computed for all E on every core (no collectives).
Activations are kept in a transposed [feature, edge] layout so every x@W
matmul maps directly onto the TensorEngine.  The masked softmax streams:
PE scoresT -> ScalarE exp (PSUM->SBUF, fp16) -> DVE mask-multiply ->
PE attn@V with a ones-column in V producing row-sums for free;
normalization happens on the small [65, e] output.

Masks are pre-transposed/tiled on the host as fp16 {0,1}; alpha.mean() is
assembled on the host from per-core partials; logits come back transposed
[16, 512] per core and are transposed on the host.
"""

import sys

sys.path.insert(0, "/opt/trn_rl_repo")

import numpy as np

import concourse.bass as bass
import concourse.bacc as bacc
import concourse.tile as tile
import concourse.mybir as mybir
from concourse.bass_utils import run_bass_kernel_spmd

F16 = mybir.dt.float16
F32 = mybir.dt.float32
AF = mybir.ActivationFunctionType
MULT = mybir.AluOpType.mult
ADD = mybir.AluOpType.add

SQRT_2_OVER_PI = 0.7978845608028654
GELU_C = 0.044715


class Cfg:
    def __init__(self, E=4096, n_cores=8, CS=3):
        self.E = E                      # total edges
        self.D = 256                    # edge feature dim
        self.H = 4                      # heads
        self.DH = 64                    # head dim
        self.NC = n_cores
        self.EL = E // n_cores          # local query rows per core
        self.NFC = E // 128             # f-chunks of 128 key rows
        self.CS = CS                    # f-chunks per softmax stripe
        self.NCLS = 16
        self.DG = 128                   # gate hidden dim
        self.n_stripes = (self.NFC + CS - 1) // CS


def _emit_core_program(nc, tc, ctx, io, cfg, rep, dbg=False):
    """Emit the whole per-core program (one repetition)."""
    E, EL, NFC, CS = cfg.E, cfg.EL, cfg.NFC, cfg.CS
    R = f"r{rep}"

    konst = ctx.enter_context(tc.tile_pool(name=f"konst{R}", bufs=1))
    ps512 = ctx.enter_context(tc.tile_pool(name=f"ps512{R}", bufs=2, space="PSUM"))
    tmp = ctx.enter_context(tc.tile_pool(name=f"tmp{R}", bufs=2))
    small = ctx.enter_context(tc.tile_pool(name=f"small{R}", bufs=1))

    def ktile(nm, shape, dtype):
        return konst.tile(shape, dtype, name=nm, tag=nm)

    def load_const(nm, shape, dtype, eng=None):
        t = ktile(nm, shape, dtype)
        (eng or nc.gpsimd).dma_start(t[:], io[nm][:])
        return t

    def load_rows(nm, src, nrow=128, nsplit=1, eng=None):
        """Load a [n*nrow, C] dram tensor as n tiles of [nrow, C]."""
        eng = eng or nc.gpsimd
        n = io[src].shape[0] // nrow
        ts = []
        for d in range(n):
            t = ktile(f"{nm}{d}", [nrow, io[src].shape[1]], io[src].dtype)
            if nsplit == 1:
                eng.dma_start(t[:], io[src][nrow * d:nrow * (d + 1), :])
            else:
                step = io[src].shape[1] // nsplit
                for i in range(nsplit):
                    eng.dma_start(
                        t[:, step * i:step * (i + 1)],
                        io[src][nrow * d:nrow * (d + 1), step * i:step * (i + 1)])
            ts.append(t)
        return ts

    # ---------------- constant loads (packed) ----------------
    xTl = load_rows("xTl", "xTl", eng=nc.sync)
    xT = load_rows("xT", "xT", nsplit=4, eng=nc.sync)

    # wpack fp16 [128, WCOLS]: all [128, x] fp16 weight tiles as column blocks
    wpack = ktile("wpack", [128, io["wpack"].shape[1]], F16)
    nc.sync.dma_start(wpack[:], io["wpack"][:])
    # bpack f32 [128, x]: per-partition bias columns
    bpack = ktile("bpack", [128, io["bpack"].shape[1]], F32)
    nc.sync.dma_start(bpack[:], io["bpack"][:])
    # rpack fp16 [1, x]: row-vector constants
    rpack = ktile("rpack", [1, io["rpack"].shape[1]], F16)
    nc.sync.dma_start(rpack[:], io["rpack"][:])
    bg2h = ktile("bg2h", [1, 1], F32)
    nc.sync.dma_start(bg2h[:], io["bg2h"][:])

    wq_off = {}
    col = 0

    def wslice(width):
        nonlocal col
        s = wpack[:, col:col + width]
        col += width
        return s

    wqkv, wv_aug, wo = [], [], []
    for a in range(2):
        wqkv.append([wslice(768) for _ in range(2)])
        wv_aug.append([wslice(260) for _ in range(2)])
        wo.append([wslice(256) for _ in range(2)])
    wg1 = [wslice(cfg.DG) for _ in range(2)]
    wc1 = [wslice(256) for _ in range(2)]
    wc2 = [wslice(cfg.NCLS) for _ in range(2)]
    wg2h = wslice(1)

    bcol = 0

    def bslice(n=1):
        nonlocal bcol
        s = bpack[:, bcol:bcol + n]
        bcol += n
        return s

    bq, bk = [], []
    for a in range(2):
        bq.append([bslice() for _ in range(2)])
        bk.append([bslice() for _ in range(2)])
    bg1 = bslice()
    bc1 = [bslice() for _ in range(2)]
    bc2_full = bslice()
    bc2 = bc2_full[0:cfg.NCLS, :]

    rcol = 0

    def rslice(n):
        nonlocal rcol
        s = rpack[:, rcol:rcol + n]
        rcol += n
        return s

    bv_aug, bo = [], []
    for a in range(2):
        bv_aug.append(rslice(260))
        bo.append(rslice(256))

    ones_row = ktile("ones_row", [1, 128], F16)
    nc.vector.memset(ones_row[:], 1.0)
    io_dbg_av = ktile("io_dbg_av", [128, 512], F32) if dbg else None

    def pstile():
        return ps512.tile([128, 512], F32, name="ps", tag="ps")

    # ---------------- gate path ----------------
    hg_ps = pstile()
    hg_psv = hg_ps[0:cfg.DG, 0:EL]
    nc.tensor.matmul(hg_psv, wg1[0], xTl[0][:], start=True, stop=False)
    nc.tensor.matmul(hg_psv, wg1[1], xTl[1][:], start=False, stop=True)

    def gelu2(z_ps, bias_ap, out_f16, P, N):
        """out = z*(1+tanh(c*(z+GELU_C*z^3))) = 2*gelu(z), z = z_ps + bias."""
        z = tmp.tile([128, 512], F32, name="gelu_z", tag="gelu_z")
        nc.vector.tensor_scalar(z[0:P, 0:N], z_ps, bias_ap, None, op0=ADD)
        u = tmp.tile([128, 512], F32, name="gelu_u", tag="gelu_t")
        nc.vector.tensor_tensor(u[0:P, 0:N], z[0:P, 0:N], z[0:P, 0:N], op=MULT)
        w = tmp.tile([128, 512], F32, name="gelu_w", tag="gelu_t")
        nc.vector.tensor_scalar(w[0:P, 0:N], u[0:P, 0:N], GELU_C, 1.0, op0=MULT, op1=ADD)
        t2 = tmp.tile([128, 512], F32, name="gelu_t2", tag="gelu_t")
        nc.vector.tensor_tensor(t2[0:P, 0:N], z[0:P, 0:N], w[0:P, 0:N], op=MULT)
        th = tmp.tile([128, 512], F32, name="gelu_th", tag="gelu_t")
        nc.scalar.activation(th[0:P, 0:N], t2[0:P, 0:N], AF.Tanh, scale=SQRT_2_OVER_PI)
        a2 = tmp.tile([128, 512], F32, name="gelu_a2", tag="gelu_t")
        nc.vector.tensor_scalar(a2[0:P, 0:N], th[0:P, 0:N], 1.0, None, op0=ADD)
        nc.vector.tensor_tensor(out_f16, z[0:P, 0:N], a2[0:P, 0:N], op=MULT)

    def stile(nm, shape, dtype):
        return small.tile(shape, dtype, name=nm, tag=nm)

    hg = stile("hg", [cfg.DG, EL], F16)  # = 2*gelu(gate pre-act)
    gelu2(hg_psv, bg1, hg[:], cfg.DG, EL)

    al_ps = pstile()
    nc.tensor.matmul(al_ps[0:1, 0:EL], wg2h, hg[:], start=True, stop=True)
    tha = stile("tha", [1, EL], F32)
    # sigmoid(l) = 0.5 + 0.5*tanh(0.5*l); wg2h has 0.25*Wg2 folded in
    nc.scalar.activation(tha[:], al_ps[0:1, 0:EL], AF.Tanh, scale=1.0, bias=bg2h[:])
    alpha = stile("alpha", [1, EL], F32)
    nc.vector.tensor_scalar(alpha[:], tha[:], 0.5, 0.5, op0=MULT, op1=ADD)
    ac = stile("ac", [1, EL], F32)
    nc.vector.tensor_scalar(ac[:], alpha[:], -1.0, 1.0, op0=MULT, op1=ADD)
    alpha16 = stile("alpha16", [1, EL], F16)
    nc.vector.tensor_copy(alpha16[:], alpha[:])
    ac16 = stile("ac16", [1, EL], F16)
    nc.vector.tensor_copy(ac16[:], ac[:])
    apart = stile("apart", [1, 1], F32)
    nc.vector.reduce_sum(apart[:], alpha[:], axis=mybir.AxisListType.X)
    nc.gpsimd.dma_start(io["alpha_part"][:], apart[:])

    ab = stile("ab", [128, EL], F32)
    ab_ps = pstile()
    nc.tensor.matmul(ab_ps[:, 0:EL], ones_row[:], alpha16[:], start=True, stop=True)
    nc.vector.tensor_copy(ab[:], ab_ps[:, 0:EL])
    acb = stile("acb", [128, EL], F32)
    acb_ps = pstile()
    nc.tensor.matmul(acb_ps[:, 0:EL], ones_row[:], ac16[:], start=True, stop=True)
    nc.vector.tensor_copy(acb[:], acb_ps[:, 0:EL])
    if dbg:
        nc.sync.dma_start(io["d_ab"][:], ab[:])
        nc.sync.dma_start(io["d_alpha"][:], alpha[:])
        nc.sync.dma_start(io["d_hg"][:], hg[:])
        nc.sync.dma_start(io["d_tha"][:], tha[:])
        hgpcp = tmp.tile([128, 512], F32, name="hgpcp", tag="hgpcp")
        nc.vector.tensor_copy(hgpcp[:, 0:EL], hg_psv)
        nc.sync.dma_start(io["d_hgps"][:], hgpcp[:, 0:EL])

    # ---------------- attention blocks ----------------
    P_sb = []  # per-attn Wo^T @ outT_norm in SBUF f32, 2 chunks of [128, EL]
    for a in range(2):
        with tc.tile_pool(name=f"attn{a}{R}", bufs=1) as apool, \
             tc.tile_pool(name=f"sc{a}{R}", bufs=2, space="PSUM") as scp, \
             tc.tile_pool(name=f"stream{a}{R}", bufs=4) as stream:

            def atile(nm, shape, dtype):
                return apool.tile(shape, dtype, name=f"{nm}_{a}", tag=nm)

            # ---- qT [256, EL] fp16 ----
            qT = [atile(f"qT{m}", [128, EL], F16) for m in range(2)]
            for m in range(2):
                ps = pstile()
                psv = ps[:, 0:EL]
                nc.tensor.matmul(psv, wqkv[a][0][:, 128 * m:128 * (m + 1)],
                                 xTl[0][:], start=True, stop=False)
                nc.tensor.matmul(psv, wqkv[a][1][:, 128 * m:128 * (m + 1)],
                                 xTl[1][:], start=False, stop=True)
                nc.vector.tensor_scalar(qT[m][:], psv, bq[a][m], None, op0=ADD)
                if dbg and a == (dbg - 1) and m == 0:
                    qpscp = tmp.tile([128, 512], F32, name="qpscp", tag="qpscp")
                    nc.vector.tensor_copy(qpscp[:, 0:EL], psv)
                    nc.sync.dma_start(io["d_qps"][:], qpscp[:, 0:EL])
            # ---- kT [256, E] fp16 ----
            kT = [atile(f"kT{m}", [128, E], F16) for m in range(2)]
            for m in range(2):
                for nn in range(E // 512):
                    ps = pstile()
                    sl = slice(512 * nn, 512 * (nn + 1))
                    nc.tensor.matmul(ps[:], wqkv[a][0][:, 256 + 128 * m:256 + 128 * (m + 1)],
                                     xT[0][:, sl], start=True, stop=False)
                    nc.tensor.matmul(ps[:], wqkv[a][1][:, 256 + 128 * m:256 + 128 * (m + 1)],
                                     xT[1][:, sl], start=False, stop=True)
                    nc.vector.tensor_scalar(kT[m][:, sl], ps[:], bk[a][m], None, op0=ADD)
            # ---- v_store [128, NFC*260] fp16 (per f-tile: 4 heads x (64 v + ones)) ----
            v_store = atile("v_store", [128, NFC * 260], F16)
            for t in range(NFC):
                ps = pstile()
                psv = ps[:, 0:260]
                fsl = slice(128 * t, 128 * (t + 1))
                nc.tensor.matmul(psv, xT[0][:, fsl], wv_aug[a][0], start=True, stop=False)
                nc.tensor.matmul(psv, xT[1][:, fsl], wv_aug[a][1], start=False, stop=False)
                nc.tensor.matmul(psv, ones_row[:], bv_aug[a], start=False, stop=True)
                nc.vector.tensor_copy(v_store[:, 260 * t:260 * (t + 1)], psv)

            # ---- heads: stripe loop, head pairs share the mask DMA ----
            outT = [atile(f"outT{m}", [128, EL], F16) for m in range(2)]
            rb = [atile(f"rb{m}", [128, EL], F32) for m in range(2)]
            for pair in range(2):
                avacc = [ps512.tile([128, 512], F32, name=f"avacc{hh}", tag="ps")
                         for hh in range(2)]
                for s in range(cfg.n_stripes):
                    w_s = min(CS, NFC - CS * s)
                    Ns = EL * w_s
                    mt = stream.tile([128, EL * CS], F16, name="mt", tag="mask")
                    for j in range(w_s):
                        nc.sync.dma_start(mt[:, EL * j:EL * (j + 1)],
                                          io[f"mask{a}"][CS * s + j])
                    for hh in range(2):
                        h = 2 * pair + hh
                        mi, po = h // 2, 64 * (h % 2)
                        sc = scp.tile([128, EL * CS], F32, name="sc", tag="sc")
                        for j in range(w_s):
                            t = CS * s + j
                            nc.tensor.matmul(
                                sc[0:128, EL * j:EL * j + EL],
                                kT[mi][po:po + 64, 128 * t:128 * (t + 1)],
                                qT[mi][po:po + 64, :],
                                start=True, stop=True)
                        pm = stream.tile([128, EL * CS], F16, name="pm", tag="pm", bufs=6)
                        nc.scalar.activation(pm[:, 0:Ns], sc[:, 0:Ns], AF.Exp, scale=0.125)
                        pmm = stream.tile([128, EL * CS], F16, name="pmm", tag="pmm", bufs=6)
                        nc.vector.tensor_tensor(pmm[:, 0:Ns], pm[:, 0:Ns], mt[:, 0:Ns], op=MULT)
                        if dbg and a == (dbg - 1) and pair == 0 and s == 0 and hh == 0:
                            nc.sync.dma_start(io["d_pmm"][:], pmm[:, 0:EL])
                            nc.sync.dma_start(io["d_pm"][:], pm[:, 0:EL])
                            nc.sync.dma_start(io["d_mt"][:], mt[:, 0:EL])
                        for j in range(w_s):
                            t = CS * s + j
                            nc.tensor.matmul(
                                avacc[hh][0:65, 0:EL],
                                v_store[:, 260 * t + 65 * h:260 * t + 65 * h + 65],
                                pmm[:, EL * j:EL * j + EL],
                                start=(t == 0), stop=(t == NFC - 1))
                # normalize: out[d, e] = avacc[d, e] * (1 / avacc[64, e])
                mi = pair
                rb_ps = scp.tile([128, EL * CS], F32, name="rb_ps", tag="sc")
                for hh in range(2):
                    rcp = stile(f"rcp{a}_{pair}_{hh}", [1, EL], F32)
                    nc.vector.reciprocal(rcp[:], avacc[hh][64:65, 0:EL])
                    rcph = stile(f"rcph{a}_{pair}_{hh}", [1, EL], F16)
                    nc.vector.tensor_copy(rcph[:], rcp[:])
                    nc.tensor.matmul(rb_ps[64 * hh:64 * hh + 64, 0:EL],
                                     ones_row[0:1, 0:64], rcph[:],
                                     start=True, stop=True)
                nc.vector.tensor_copy(rb[mi][:], rb_ps[:, 0:EL])
                for hh in range(2):
                    po = 64 * hh
                    nc.vector.tensor_tensor(outT[mi][po:po + 64, :],
                                            avacc[hh][0:64, 0:EL],
                                            rb[mi][po:po + 64, :], op=MULT)
                    if dbg and a == (dbg - 1) and h == 0:
                        nc.vector.tensor_copy(io_dbg_av[0:65, 0:EL], avacc[hh][0:65, 0:EL])
                        nc.sync.dma_start(io["d_avacc"][:], io_dbg_av[0:65, 0:EL])
                        nc.sync.dma_start(io["d_rb"][:], rb[0][:])

            # ---- P = Wo^T @ outT  (f32 in SBUF; bias folded at blend) ----
            Pm = []
            cb = acb if a == 0 else ab
            for m in range(2):
                ps = pstile()
                psv = ps[:, 0:EL]
                nc.tensor.matmul(psv, wo[a][0][:, 128 * m:128 * (m + 1)], outT[0][:],
                                 start=True, stop=False)
                nc.tensor.matmul(psv, wo[a][1][:, 128 * m:128 * (m + 1)], outT[1][:],
                                 start=False, stop=True)
                p_sb = tmp.tile([128, 512], F32, name=f"P{a}_{m}", tag=f"P{a}_{m}")
                nc.vector.tensor_tensor(p_sb[:, 0:EL], psv, cb[:], op=MULT)
                Pm.append(p_sb)
            if dbg and a == (dbg - 1):
                nc.sync.dma_start(io["d_outT"][:], outT[0][:])
                nc.sync.dma_start(io["d_P"][:], Pm[0][:, 0:EL])
                nc.sync.dma_start(io["d_qT"][:], qT[0][:])
                nc.sync.dma_start(io["d_kT"][:], kT[0][:, 0:EL])
                nc.sync.dma_start(io["d_v"][:], v_store[:, 0:260])
            P_sb.append(Pm)

    # ---------------- blend + classifier ----------------
    blended = [stile(f"blend{m}", [128, EL], F16) for m in range(2)]
    for m in range(2):
        bb = pstile()
        bbv = bb[:, 0:EL]
        nc.tensor.matmul(bbv, bo[0][0:1, 128 * m:128 * (m + 1)], ac16[:], start=True, stop=False)
        nc.tensor.matmul(bbv, bo[1][0:1, 128 * m:128 * (m + 1)], alpha16[:], start=False, stop=True)
        t3 = tmp.tile([128, 512], F32, name="bl_t3", tag="bl_t3")
        nc.vector.tensor_tensor(t3[:, 0:EL], P_sb[0][m][:, 0:EL], P_sb[1][m][:, 0:EL], op=ADD)
        nc.vector.tensor_tensor(blended[m][:], t3[:, 0:EL], bbv, op=ADD)
        if dbg and m == 0:
            bbcp = tmp.tile([128, 512], F32, name="bbcp", tag="bbcp")
            nc.vector.tensor_copy(bbcp[:, 0:EL], bbv)
            nc.sync.dma_start(io["d_bb"][:], bbcp[:, 0:EL])
            nc.sync.dma_start(io["d_t3"][:], t3[:, 0:EL])
            nc.sync.dma_start(io["d_blended"][:], blended[0][:])
            nc.sync.dma_start(io["d_t1"][:], t1[:, 0:EL])
            nc.sync.dma_start(io["d_t2"][:], t2[:, 0:EL])
            nc.sync.dma_start(io["d_ab2"][:], ab[:])
            nc.sync.dma_start(io["d_acb2"][:], acb[:])
            nc.sync.dma_start(io["d_P00"][:], P_sb[0][0][:, 0:EL])
            nc.sync.dma_start(io["d_P10"][:], P_sb[1][0][:, 0:EL])

    chT = [stile(f"ch{m}", [128, EL], F16) for m in range(2)]
    for m in range(2):
        ps = pstile()
        psv = ps[:, 0:EL]
        nc.tensor.matmul(psv, wc1[0][:, 128 * m:128 * (m + 1)], blended[0][:], start=True, stop=False)
        nc.tensor.matmul(psv, wc1[1][:, 128 * m:128 * (m + 1)], blended[1][:], start=False, stop=True)
        gelu2(psv, bc1[m], chT[m][:], 128, EL)
        if dbg and m == 0:
            zcp = tmp.tile([128, 512], F32, name="zcp", tag="zcp")
            nc.vector.tensor_copy(zcp[:, 0:EL], psv)
            nc.sync.dma_start(io["d_zc"][:], zcp[:, 0:EL])
            nc.sync.dma_start(io["d_ch"][:], chT[0][:])
    # logitsT = Wc2'^T @ chT + bc2, with the 0.5 of gelu2 folded into Wc2'
    lp = pstile()
    lpv = lp[0:cfg.NCLS, 0:EL]
    nc.tensor.matmul(lpv, wc2[0], chT[0][:], start=True, stop=False)
    nc.tensor.matmul(lpv, wc2[1], chT[1][:], start=False, stop=True)
    logT = stile("logT", [cfg.NCLS, EL], F32)
    nc.vector.tensor_scalar(logT[:], lpv, bc2, None, op0=ADD)
    nc.sync.dma_start(io["logitsT"][:], logT[:])


def build_nc(cfg, reps=1, dbg=False):
    nc = bacc.Bacc("TRN2", target_bir_lowering=False, debug=False,
                   num_devices=cfg.NC)
    io = {}

    def din(name, shape, dtype):
        io[name] = nc.dram_tensor(name, shape, dtype, kind="ExternalInput").ap()

    E = cfg.E
    din("xT", [256, E], F16)
    din("xTl", [256, cfg.EL], F16)
    for a in range(2):
        din(f"mask{a}", [cfg.NFC, 128, cfg.EL], F16)
    WCOLS = 2 * (2 * 768 + 2 * 260 + 2 * 256) + 2 * cfg.DG + 2 * 256 + 2 * cfg.NCLS + 1
    din("wpack", [128, WCOLS], F16)
    din("bpack", [128, 12], F32)
    din("rpack", [1, 2 * (260 + 256)], F16)
    din("bg2h", [1, 1], F32)
    io["logitsT"] = nc.dram_tensor("logitsT", [cfg.NCLS, cfg.EL], F32,
                                   kind="ExternalOutput").ap()
    io["alpha_part"] = nc.dram_tensor("alpha_part", [1, 1], F32,
                                      kind="ExternalOutput").ap()
    if dbg:
        for nm, shape, dt in [
            ("d_ab", [128, cfg.EL], F32), ("d_alpha", [1, cfg.EL], F32),
            ("d_pmm", [128, cfg.EL], F16), ("d_pm", [128, cfg.EL], F16),
            ("d_mt", [128, cfg.EL], F16), ("d_avacc", [65, cfg.EL], F32),
            ("d_rb", [128, cfg.EL], F32), ("d_outT", [128, cfg.EL], F16),
            ("d_P", [128, cfg.EL], F32), ("d_qT", [128, cfg.EL], F16),
            ("d_kT", [128, cfg.EL], F16), ("d_v", [128, 260], F16),
            ("d_bb", [128, cfg.EL], F32), ("d_t3", [128, cfg.EL], F32),
            ("d_blended", [128, cfg.EL], F16), ("d_zc", [128, cfg.EL], F32),
            ("d_ch", [128, cfg.EL], F16),
            ("d_t1", [128, cfg.EL], F32), ("d_t2", [128, cfg.EL], F32),
            ("d_ab2", [128, cfg.EL], F32), ("d_acb2", [128, cfg.EL], F32),
            ("d_P00", [128, cfg.EL], F32), ("d_P10", [128, cfg.EL], F32),
            ("d_qps", [128, cfg.EL], F32),
            ("d_hg", [128, cfg.EL], F16), ("d_tha", [1, cfg.EL], F32),
            ("d_hgps", [128, cfg.EL], F32),
        ]:
            io[nm] = nc.dram_tensor(nm, shape, dt, kind="ExternalOutput").ap()

    from contextlib import ExitStack
    with tile.TileContext(nc) as tc:
        for rep in range(reps):
            with ExitStack() as ctx:
                _emit_core_program(nc, tc, ctx, io, cfg, rep, dbg=dbg)
    nc.compile()
    return nc


def host_prep(cfg, inputs):
    """Build the per-core input maps from the full problem inputs."""
    E, EL, NFC = cfg.E, cfg.EL, cfg.NFC
    f16 = np.float16
    x = np.asarray(inputs["edge_features"], np.float32)
    xT = np.ascontiguousarray(x.T).astype(f16)
    shared = {"xT": xT}
    wcols, bcols, rcols = [], [], []
    for a, (wq, bqv, wo_, bo_) in enumerate([
        (inputs["Wqkv1"], inputs["bqkv1"], inputs["Wo1"], inputs["bo1"]),
        (inputs["Wqkv2"], inputs["bqkv2"], inputs["Wo2"], inputs["bo2"]),
    ]):
        wq = np.asarray(wq, np.float32)
        bqv = np.asarray(bqv, np.float32)
        wo_ = np.asarray(wo_, np.float32)
        bo_ = np.asarray(bo_, np.float32)
        wv_aug = np.zeros((256, 260), np.float32)
        bv_aug = np.zeros((1, 260), np.float32)
        for h in range(4):
            wv_aug[:, 65 * h:65 * h + 64] = wq[:, 512 + 64 * h:512 + 64 * (h + 1)]
            bv_aug[0, 65 * h:65 * h + 64] = bqv[512 + 64 * h:512 + 64 * (h + 1)]
            bv_aug[0, 65 * h + 64] = 1.0
        wcols += [wq[0:128], wq[128:256], wv_aug[0:128], wv_aug[128:256],
                  wo_[0:128], wo_[128:256]]
        bcols += [bqv[0:128, None], bqv[128:256, None],
                  bqv[256:384, None], bqv[384:512, None]]
        rcols += [bv_aug, bo_.reshape(1, 256)]
    wg1 = np.asarray(inputs["Wg1"], np.float32)
    wc1 = np.asarray(inputs["Wc1"], np.float32)
    wc2h = 0.5 * np.asarray(inputs["Wc2"], np.float32)
    wg2h = 0.25 * np.asarray(inputs["Wg2"], np.float32)
    wcols += [wg1[0:128], wg1[128:256], wc1[0:128], wc1[128:256],
              wc2h[0:128], wc2h[128:256], wg2h[0:128]]
    bc2c = np.zeros((128, 1), np.float32)
    bc2c[0:16, 0] = np.asarray(inputs["bc2"], np.float32)
    bcols += [np.asarray(inputs["bg1"], np.float32).reshape(128, 1),
              np.asarray(inputs["bc1"], np.float32)[0:128, None],
              np.asarray(inputs["bc1"], np.float32)[128:256, None],
              bc2c]
    shared["wpack"] = np.ascontiguousarray(np.concatenate(wcols, axis=1)).astype(f16)
    shared["bpack"] = np.ascontiguousarray(np.concatenate(bcols, axis=1)).astype(np.float32)
    shared["rpack"] = np.ascontiguousarray(np.concatenate(rcols, axis=1)).astype(f16)
    shared["bg2h"] = (0.5 * np.asarray(inputs["bg2"], np.float32)).reshape(1, 1)

    adjT = [np.asarray(inputs["adj1"]).T, np.asarray(inputs["adj2"]).T]
    in_maps = []
    for c in range(cfg.NC):
        m = dict(shared)
        sl = slice(EL * c, EL * (c + 1))
        m["xTl"] = np.ascontiguousarray(shared["xT"][:, sl])
        for a in range(2):
            mt = adjT[a][:, sl].astype(f16)           # [E, EL]
            m[f"mask{a}"] = np.ascontiguousarray(mt.reshape(NFC, 128, EL))
        in_maps.append(m)
    return in_maps


_CACHE = {}


def _get_nc(cfg, reps=1):
    key = (cfg.E, cfg.NC, cfg.CS, reps)
    if key not in _CACHE:
        _CACHE[key] = build_nc(cfg, reps)
    return _CACHE[key]


def kernel(**inputs):
    cfg = Cfg()
    nc = _get_nc(cfg)
    in_maps = host_prep(cfg, inputs)
    res = run_bass_kernel_spmd(nc, in_maps, core_ids=list(range(cfg.NC)))
    logits = np.concatenate(
        [np.ascontiguousarray(res.results[c]["logitsT"].T) for c in range(cfg.NC)],
        axis=0)
    alpha_mean = np.float32(
        sum(float(res.results[c]["alpha_part"][0, 0]) for c in range(cfg.NC))
        / cfg.E)
    return logits.astype(np.float32), alpha_mean


# revision 39
# speedup vs baseline: 6.8779x; 1.5595x over previous
"""Trainium2 distributed kernel for nn_AdaptiveHopModel (gnn_message_passing).

Model: two masked edge-attention blocks over E=4096 edges (D=256, H=4), an
adaptive per-edge hop gate alpha = sigmoid(MLP(x)), blended output fed
through a small classifier; returns (logits [E,16], alpha.mean()).

Sharding: query-edge rows (E) split across 8 NeuronCores (512 rows/core),
weights replicated, K/V computed for all E on every core (no collectives —
measured AllGather cost in this environment dwarfs the replicated compute).
Activations are kept in a transposed [feature, edge] layout so every x@W
matmul maps directly onto the TensorEngine.  The masked softmax streams:
PE scoresT -> ScalarE exp (PSUM->SBUF, fp16) -> DVE mask-multiply ->
PE attn@V with a ones-column in V producing row-sums for free;
normalization happens on the small [65, e] output.

Masks are pre-transposed/tiled on the host as fp16 {0,1}; weights arrive
packed into three wide tensors (3 DMAs); alpha.mean() is assembled on the
host from per-core partials; logits come back transposed [16, 512] per
core and are transposed on the host.
"""

import sys

sys.path.insert(0, "/opt/trn_rl_repo")

import numpy as np

import concourse.bass as bass
import concourse.bacc as bacc
import concourse.tile as tile
import concourse.mybir as mybir
from concourse.bass_utils import run_bass_kernel_spmd

F16 = mybir.dt.float16
F32 = mybir.dt.float32
AF = mybir.ActivationFunctionType
MULT = mybir.AluOpType.mult
ADD = mybir.AluOpType.add

SQRT_2_OVER_PI = 0.7978845608028654
GELU_C = 0.044715


class Cfg:
    def __init__(self, E=4096, n_cores=8, CS=2):
        self.E = E                      # total edges
        self.D = 256                    # edge feature dim
        self.H = 4                      # heads
        self.DH = 64                    # head dim
        self.NC = n_cores
        self.EL = E // n_cores          # local query rows per core
        self.NFC = E // 128             # f-chunks of 128 key rows
        self.CS = CS                    # f-chunks per softmax stripe
        self.NCLS = 16
        self.DG = 128                   # gate hidden dim
        self.n_stripes = (self.NFC + CS - 1) // CS


def _emit_core_program(nc, tc, ctx, io, cfg, rep, dbg=False):
    """Emit the whole per-core program (one repetition)."""
    E, EL, NFC, CS = cfg.E, cfg.EL, cfg.NFC, cfg.CS
    R = f"r{rep}"

    konst = ctx.enter_context(tc.tile_pool(name=f"konst{R}", bufs=1))
    ps512 = ctx.enter_context(tc.tile_pool(name=f"ps512{R}", bufs=2, space="PSUM"))
    tmp = ctx.enter_context(tc.tile_pool(name=f"tmp{R}", bufs=2))
    small = ctx.enter_context(tc.tile_pool(name=f"small{R}", bufs=1))
    apool = ctx.enter_context(tc.tile_pool(name=f"attn{R}", bufs=1))
    scp = ctx.enter_context(tc.tile_pool(name=f"scp{R}", bufs=2, space="PSUM"))
    stream = ctx.enter_context(tc.tile_pool(name=f"stream{R}", bufs=3))

    def ktile(nm, shape, dtype):
        return konst.tile(shape, dtype, name=nm, tag=nm)

    def load_rows(nm, src, nrow=128, nsplit=1, eng=None):
        eng = eng or nc.sync
        n = io[src].shape[0] // nrow
        ts = []
        for d in range(n):
            t = ktile(f"{nm}{d}", [nrow, io[src].shape[1]], io[src].dtype)
            if nsplit == 1:
                eng.dma_start(t[:], io[src][nrow * d:nrow * (d + 1), :])
            else:
                step = io[src].shape[1] // nsplit
                for i in range(nsplit):
                    eng.dma_start(
                        t[:, step * i:step * (i + 1)],
                        io[src][nrow * d:nrow * (d + 1), step * i:step * (i + 1)])
            ts.append(t)
        return ts

    # ---------------- constant loads (packed) ----------------
    xTl = load_rows("xTl", "xTl")
    xpool_cm = tc.tile_pool(name=f"xpool{R}", bufs=1)
    xpool = xpool_cm.__enter__()
    xT = []
    for d in range(2):
        t = xpool.tile([128, E], F16, name=f"xT{d}", tag=f"xT{d}")
        for i in range(4):
            nc.sync.dma_start(t[:, (E // 4) * i:(E // 4) * (i + 1)],
                              io["xT"][128 * d:128 * (d + 1),
                                       (E // 4) * i:(E // 4) * (i + 1)])
        xT.append(t)

    wpack = ktile("wpack", [128, io["wpack"].shape[1]], F16)
    nc.sync.dma_start(wpack[:], io["wpack"][:])
    bpack = ktile("bpack", [128, io["bpack"].shape[1]], F32)
    nc.sync.dma_start(bpack[:], io["bpack"][:])
    rpack = ktile("rpack", [1, io["rpack"].shape[1]], F16)
    nc.sync.dma_start(rpack[:], io["rpack"][:])
    bg2h = ktile("bg2h", [1, 1], F32)
    nc.sync.dma_start(bg2h[:], io["bg2h"][:])

    col = 0

    def wslice(width):
        nonlocal col
        s = wpack[:, col:col + width]
        col += width
        return s

    wqkv, wv_aug, wo = [], [], []
    for a in range(2):
        wqkv.append([wslice(768) for _ in range(2)])
        wv_aug.append([wslice(260) for _ in range(2)])
        wo.append([wslice(256) for _ in range(2)])
    wg1 = [wslice(cfg.DG) for _ in range(2)]
    wc1 = [wslice(256) for _ in range(2)]
    wc2 = [wslice(cfg.NCLS) for _ in range(2)]
    wg2h = wslice(1)

    bcol = 0

    def bslice(n=1):
        nonlocal bcol
        s = bpack[:, bcol:bcol + n]
        bcol += n
        return s

    bq, bk = [], []
    for a in range(2):
        bq.append([bslice() for _ in range(2)])
        bk.append([bslice() for _ in range(2)])
    bg1 = bslice()
    bc1 = [bslice() for _ in range(2)]
    bc2 = bslice()[0:cfg.NCLS, :]

    rcol = 0

    def rslice(n):
        nonlocal rcol
        s = rpack[:, rcol:rcol + n]
        rcol += n
        return s

    bv_aug, bo = [], []
    for a in range(2):
        bv_aug.append(rslice(260))
        bo.append(rslice(256))

    ones_row = ktile("ones_row", [1, 128], F16)
    nc.vector.memset(ones_row[:], 1.0)

    def pstile():
        return ps512.tile([128, 512], F32, name="ps", tag="ps")

    def gelu2(z_ps, bias_ap, out_f16, P, N):
        """out = z*(1+tanh(c*(z+GELU_C*z^3))) = 2*gelu(z), z = z_ps + bias."""
        z = tmp.tile([128, 512], F32, name="gelu_z", tag="gelu_z")
        nc.vector.tensor_scalar(z[0:P, 0:N], z_ps, bias_ap, None, op0=ADD)
        u = tmp.tile([128, 512], F32, name="gelu_u", tag="gelu_t")
        nc.vector.tensor_tensor(u[0:P, 0:N], z[0:P, 0:N], z[0:P, 0:N], op=MULT)
        w = tmp.tile([128, 512], F32, name="gelu_w", tag="gelu_t")
        nc.vector.tensor_scalar(w[0:P, 0:N], u[0:P, 0:N], GELU_C, 1.0, op0=MULT, op1=ADD)
        t2 = tmp.tile([128, 512], F32, name="gelu_t2", tag="gelu_t")
        nc.vector.tensor_tensor(t2[0:P, 0:N], z[0:P, 0:N], w[0:P, 0:N], op=MULT)
        th = tmp.tile([128, 512], F32, name="gelu_th", tag="gelu_t")
        nc.scalar.activation(th[0:P, 0:N], t2[0:P, 0:N], AF.Tanh, scale=SQRT_2_OVER_PI)
        a2 = tmp.tile([128, 512], F32, name="gelu_a2", tag="gelu_t")
        nc.vector.tensor_scalar(a2[0:P, 0:N], th[0:P, 0:N], 1.0, None, op0=ADD)
        nc.vector.tensor_tensor(out_f16, z[0:P, 0:N], a2[0:P, 0:N], op=MULT)

    def stile(nm, shape, dtype):
        return small.tile(shape, dtype, name=nm, tag=nm)

    def atile(nm, shape, dtype):
        return apool.tile(shape, dtype, name=nm, tag=nm)

    # ---------------- q/k/v for both attentions ----------------
    qTs, kTs, vss = [], [], []
    for a in range(2):
        qT = [atile(f"qT{m}_{a}", [128, EL], F16) for m in range(2)]
        for m in range(2):
            ps = pstile()
            psv = ps[:, 0:EL]
            nc.tensor.matmul(psv, wqkv[a][0][:, 128 * m:128 * (m + 1)],
                             xTl[0][:], start=True, stop=False)
            nc.tensor.matmul(psv, wqkv[a][1][:, 128 * m:128 * (m + 1)],
                             xTl[1][:], start=False, stop=True)
            nc.vector.tensor_scalar(qT[m][:], psv, bq[a][m], None, op0=ADD)
        kT = [atile(f"kT{m}_{a}", [128, E], F16) for m in range(2)]
        for m in range(2):
            for nn in range(E // 512):
                ps = pstile()
                sl = slice(512 * nn, 512 * (nn + 1))
                nc.tensor.matmul(ps[:], wqkv[a][0][:, 256 + 128 * m:256 + 128 * (m + 1)],
                                 xT[0][:, sl], start=True, stop=False)
                nc.tensor.matmul(ps[:], wqkv[a][1][:, 256 + 128 * m:256 + 128 * (m + 1)],
                                 xT[1][:, sl], start=False, stop=True)
                nc.vector.tensor_scalar(kT[m][:, sl], ps[:], bk[a][m], None, op0=ADD)
        v_store = atile(f"v_store_{a}", [128, NFC * 260], F16)
        for t in range(NFC):
            ps = pstile()
            psv = ps[:, 0:260]
            fsl = slice(128 * t, 128 * (t + 1))
            nc.tensor.matmul(psv, xT[0][:, fsl], wv_aug[a][0], start=True, stop=False)
            nc.tensor.matmul(psv, xT[1][:, fsl], wv_aug[a][1], start=False, stop=False)
            nc.tensor.matmul(psv, ones_row[:], bv_aug[a], start=False, stop=True)
            nc.vector.tensor_copy(v_store[:, 260 * t:260 * (t + 1)], psv)
        qTs.append(qT)
        kTs.append(kT)
        vss.append(v_store)

    xpool_cm.__exit__(None, None, None)

    # ---------------- gate path ----------------
    hg_ps = pstile()
    hg_psv = hg_ps[0:cfg.DG, 0:EL]
    nc.tensor.matmul(hg_psv, wg1[0], xTl[0][:], start=True, stop=False)
    nc.tensor.matmul(hg_psv, wg1[1], xTl[1][:], start=False, stop=True)
    hg = stile("hg", [cfg.DG, EL], F16)  # = 2*gelu(gate pre-act)
    gelu2(hg_psv, bg1, hg[:], cfg.DG, EL)
    al_ps = pstile()
    nc.tensor.matmul(al_ps[0:1, 0:EL], wg2h, hg[:], start=True, stop=True)
    tha = stile("tha", [1, EL], F32)
    # sigmoid(l) = 0.5 + 0.5*tanh(0.5*l); wg2h has 0.25*Wg2 folded in
    nc.scalar.activation(tha[:], al_ps[0:1, 0:EL], AF.Tanh, scale=1.0, bias=bg2h[:])
    alpha = stile("alpha", [1, EL], F32)
    nc.vector.tensor_scalar(alpha[:], tha[:], 0.5, 0.5, op0=MULT, op1=ADD)
    ac = stile("ac", [1, EL], F32)
    nc.vector.tensor_scalar(ac[:], alpha[:], -1.0, 1.0, op0=MULT, op1=ADD)
    alpha16 = stile("alpha16", [1, EL], F16)
    nc.vector.tensor_copy(alpha16[:], alpha[:])
    ac16 = stile("ac16", [1, EL], F16)
    nc.vector.tensor_copy(ac16[:], ac[:])
    apart = stile("apart", [1, 1], F32)
    nc.vector.reduce_sum(apart[:], alpha[:], axis=mybir.AxisListType.X)
    nc.gpsimd.dma_start(io["alpha_part"][:], apart[:])

    ab = stile("ab", [128, EL], F32)
    ab_ps = pstile()
    nc.tensor.matmul(ab_ps[:, 0:EL], ones_row[:], alpha16[:], start=True, stop=True)
    nc.vector.tensor_copy(ab[:], ab_ps[:, 0:EL])
    acb = stile("acb", [128, EL], F32)
    acb_ps = pstile()
    nc.tensor.matmul(acb_ps[:, 0:EL], ones_row[:], ac16[:], start=True, stop=True)
    nc.vector.tensor_copy(acb[:], acb_ps[:, 0:EL])

    # ---------------- attention head loops ----------------
    stream = ctx.enter_context(tc.tile_pool(name=f"stream{R}", bufs=3))
    P_sb = []
    for a in range(2):
        qT, kT, v_store = qTs[a], kTs[a], vss[a]
        outT = [atile(f"outT{m}_{a}", [128, EL], F16) for m in range(2)]
        rb = [atile(f"rb{m}_{a}", [128, EL], F32) for m in range(2)]
        for pair in range(2):
            avacc = [ps512.tile([128, 512], F32, name=f"avacc{hh}", tag="av")
                     for hh in range(2)]
            for s in range(cfg.n_stripes):
                w_s = min(CS, NFC - CS * s)
                Ns = EL * w_s
                mt = stream.tile([128, EL * CS], F16, name="mt", tag="mask")
                for j in range(w_s):
                    nc.sync.dma_start(mt[:, EL * j:EL * (j + 1)],
                                      io[f"mask{a}"][CS * s + j])
                for hh in range(2):
                    h = 2 * pair + hh
                    mi, po = h // 2, 64 * (h % 2)
                    sc = scp.tile([128, EL * CS], F32, name="sc", tag="sc")
                    for j in range(w_s):
                        t = CS * s + j
                        nc.tensor.matmul(
                            sc[0:128, EL * j:EL * j + EL],
                            kT[mi][po:po + 64, 128 * t:128 * (t + 1)],
                            qT[mi][po:po + 64, :],
                            start=True, stop=True)
                    pm = stream.tile([128, EL * CS], F16, name="pm", tag="pm", bufs=4)
                    nc.scalar.activation(pm[:, 0:Ns], sc[:, 0:Ns], AF.Exp, scale=0.125)
                    pmm = stream.tile([128, EL * CS], F16, name="pmm", tag="pmm", bufs=4)
                    nc.vector.tensor_tensor(pmm[:, 0:Ns], pm[:, 0:Ns], mt[:, 0:Ns], op=MULT)
                    for j in range(w_s):
                        t = CS * s + j
                        nc.tensor.matmul(
                            avacc[hh][0:65, 0:EL],
                            v_store[:, 260 * t + 65 * h:260 * t + 65 * h + 65],
                            pmm[:, EL * j:EL * j + EL],
                            start=(t == 0), stop=(t == NFC - 1))
            # normalize: out[d, e] = avacc[d, e] * (1 / avacc[64, e])
            mi = pair
            rb_ps = scp.tile([128, EL * CS], F32, name="rb_ps", tag="sc")
            for hh in range(2):
                rcp = stile(f"rcp{a}_{pair}_{hh}", [1, EL], F32)
                nc.vector.reciprocal(rcp[:], avacc[hh][64:65, 0:EL])
                rcph = stile(f"rcph{a}_{pair}_{hh}", [1, EL], F16)
                nc.vector.tensor_copy(rcph[:], rcp[:])
                nc.tensor.matmul(rb_ps[64 * hh:64 * hh + 64, 0:EL],
                                 ones_row[0:1, 0:64], rcph[:],
                                 start=True, stop=True)
            nc.vector.tensor_copy(rb[mi][:], rb_ps[:, 0:EL])
            for hh in range(2):
                po = 64 * hh
                nc.vector.tensor_tensor(outT[mi][po:po + 64, :],
                                        avacc[hh][0:64, 0:EL],
                                        rb[mi][po:po + 64, :], op=MULT)

        # ---- P = Wo^T @ outT, premultiplied by blend coefficient ----
        Pm = []
        cb = acb if a == 0 else ab
        for m in range(2):
            ps = pstile()
            psv = ps[:, 0:EL]
            nc.tensor.matmul(psv, wo[a][0][:, 128 * m:128 * (m + 1)], outT[0][:],
                             start=True, stop=False)
            nc.tensor.matmul(psv, wo[a][1][:, 128 * m:128 * (m + 1)], outT[1][:],
                             start=False, stop=True)
            p_sb = tmp.tile([128, 512], F32, name=f"P{a}_{m}", tag=f"P{a}_{m}")
            nc.vector.tensor_tensor(p_sb[:, 0:EL], psv, cb[:], op=MULT)
            Pm.append(p_sb)
        P_sb.append(Pm)

    # ---------------- blend + classifier ----------------
    blended = [stile(f"blend{m}", [128, EL], F16) for m in range(2)]
    for m in range(2):
        bb = pstile()
        bbv = bb[:, 0:EL]
        nc.tensor.matmul(bbv, bo[0][0:1, 128 * m:128 * (m + 1)], ac16[:], start=True, stop=False)
        nc.tensor.matmul(bbv, bo[1][0:1, 128 * m:128 * (m + 1)], alpha16[:], start=False, stop=True)
        t3 = tmp.tile([128, 512], F32, name="bl_t3", tag="bl_t3")
        nc.vector.tensor_tensor(t3[:, 0:EL], P_sb[0][m][:, 0:EL], P_sb[1][m][:, 0:EL], op=ADD)
        nc.vector.tensor_tensor(blended[m][:], t3[:, 0:EL], bbv, op=ADD)

    chT = [stile(f"ch{m}", [128, EL], F16) for m in range(2)]
    for m in range(2):
        ps = pstile()
        psv = ps[:, 0:EL]
        nc.tensor.matmul(psv, wc1[0][:, 128 * m:128 * (m + 1)], blended[0][:], start=True, stop=False)
        nc.tensor.matmul(psv, wc1[1][:, 128 * m:128 * (m + 1)], blended[1][:], start=False, stop=True)
        gelu2(psv, bc1[m], chT[m][:], 128, EL)
    # logitsT = Wc2'^T @ chT + bc2, with the 0.5 of gelu2 folded into Wc2'
    lp = pstile()
    lpv = lp[0:cfg.NCLS, 0:EL]
    nc.tensor.matmul(lpv, wc2[0], chT[0][:], start=True, stop=False)
    nc.tensor.matmul(lpv, wc2[1], chT[1][:], start=False, stop=True)
    logT = stile("logT", [cfg.NCLS, EL], F32)
    nc.vector.tensor_scalar(logT[:], lpv, bc2, None, op0=ADD)
    nc.sync.dma_start(io["logitsT"][:], logT[:])


def build_nc(cfg, reps=1, dbg=False):
    nc = bacc.Bacc("TRN2", target_bir_lowering=False, debug=False,
                   num_devices=cfg.NC)
    io = {}

    def din(name, shape, dtype):
        io[name] = nc.dram_tensor(name, shape, dtype, kind="ExternalInput").ap()

    E = cfg.E
    din("xT", [256, E], F16)
    din("xTl", [256, cfg.EL], F16)
    for a in range(2):
        din(f"mask{a}", [cfg.NFC, 128, cfg.EL], F16)
    WCOLS = 2 * (2 * 768 + 2 * 260 + 2 * 256) + 2 * cfg.DG + 2 * 256 + 2 * cfg.NCLS + 1
    din("wpack", [128, WCOLS], F16)
    din("bpack", [128, 12], F32)
    din("rpack", [1, 2 * (260 + 256)], F16)
    din("bg2h", [1, 1], F32)
    io["logitsT"] = nc.dram_tensor("logitsT", [cfg.NCLS, cfg.EL], F32,
                                   kind="ExternalOutput").ap()
    io["alpha_part"] = nc.dram_tensor("alpha_part", [1, 1], F32,
                                      kind="ExternalOutput").ap()
    if dbg:
        for nm, shape, dt in [
            ("d_ab", [128, cfg.EL], F32), ("d_alpha", [1, cfg.EL], F32),
            ("d_pmm", [128, cfg.EL], F16), ("d_pm", [128, cfg.EL], F16),
            ("d_mt", [128, cfg.EL], F16), ("d_avacc", [65, cfg.EL], F32),
            ("d_rb", [128, cfg.EL], F32), ("d_outT", [128, cfg.EL], F16),
            ("d_P", [128, cfg.EL], F32), ("d_qT", [128, cfg.EL], F16),
            ("d_kT", [128, cfg.EL], F16), ("d_v", [128, 260], F16),
            ("d_bb", [128, cfg.EL], F32), ("d_t3", [128, cfg.EL], F32),
            ("d_blended", [128, cfg.EL], F16), ("d_zc", [128, cfg.EL], F32),
            ("d_ch", [128, cfg.EL], F16),
            ("d_t1", [128, cfg.EL], F32), ("d_t2", [128, cfg.EL], F32),
            ("d_ab2", [128, cfg.EL], F32), ("d_acb2", [128, cfg.EL], F32),
            ("d_P00", [128, cfg.EL], F32), ("d_P10", [128, cfg.EL], F32),
            ("d_qps", [128, cfg.EL], F32),
            ("d_hg", [128, cfg.EL], F16), ("d_tha", [1, cfg.EL], F32),
            ("d_hgps", [128, cfg.EL], F32),
        ]:
            io[nm] = nc.dram_tensor(nm, shape, dt, kind="ExternalOutput").ap()

    from contextlib import ExitStack
    with tile.TileContext(nc) as tc:
        for rep in range(reps):
            with ExitStack() as ctx:
                _emit_core_program(nc, tc, ctx, io, cfg, rep, dbg=dbg)
    nc.compile()
    return nc


def host_prep(cfg, inputs):
    """Build the per-core input maps from the full problem inputs."""
    E, EL, NFC = cfg.E, cfg.EL, cfg.NFC
    f16 = np.float16
    x = np.asarray(inputs["edge_features"], np.float32)
    xT = np.ascontiguousarray(x.T).astype(f16)
    shared = {"xT": xT}
    wcols, bcols, rcols = [], [], []
    for a, (wq, bqv, wo_, bo_) in enumerate([
        (inputs["Wqkv1"], inputs["bqkv1"], inputs["Wo1"], inputs["bo1"]),
        (inputs["Wqkv2"], inputs["bqkv2"], inputs["Wo2"], inputs["bo2"]),
    ]):
        wq = np.asarray(wq, np.float32)
        bqv = np.asarray(bqv, np.float32)
        wo_ = np.asarray(wo_, np.float32)
        bo_ = np.asarray(bo_, np.float32)
        wv_aug = np.zeros((256, 260), np.float32)
        bv_aug = np.zeros((1, 260), np.float32)
        for h in range(4):
            wv_aug[:, 65 * h:65 * h + 64] = wq[:, 512 + 64 * h:512 + 64 * (h + 1)]
            bv_aug[0, 65 * h:65 * h + 64] = bqv[512 + 64 * h:512 + 64 * (h + 1)]
            bv_aug[0, 65 * h + 64] = 1.0
        wcols += [wq[0:128], wq[128:256], wv_aug[0:128], wv_aug[128:256],
                  wo_[0:128], wo_[128:256]]
        bcols += [bqv[0:128, None], bqv[128:256, None],
                  bqv[256:384, None], bqv[384:512, None]]
        rcols += [bv_aug, bo_.reshape(1, 256)]
    wg1 = np.asarray(inputs["Wg1"], np.float32)
    wc1 = np.asarray(inputs["Wc1"], np.float32)
    wc2h = 0.5 * np.asarray(inputs["Wc2"], np.float32)
    wg2h = 0.25 * np.asarray(inputs["Wg2"], np.float32)
    wcols += [wg1[0:128], wg1[128:256], wc1[0:128], wc1[128:256],
              wc2h[0:128], wc2h[128:256], wg2h[0:128]]
    bc2c = np.zeros((128, 1), np.float32)
    bc2c[0:16, 0] = np.asarray(inputs["bc2"], np.float32)
    bcols += [np.asarray(inputs["bg1"], np.float32).reshape(128, 1),
              np.asarray(inputs["bc1"], np.float32)[0:128, None],
              np.asarray(inputs["bc1"], np.float32)[128:256, None],
              bc2c]
    shared["wpack"] = np.ascontiguousarray(np.concatenate(wcols, axis=1)).astype(f16)
    shared["bpack"] = np.ascontiguousarray(np.concatenate(bcols, axis=1)).astype(np.float32)
    shared["rpack"] = np.ascontiguousarray(np.concatenate(rcols, axis=1)).astype(f16)
    shared["bg2h"] = (0.5 * np.asarray(inputs["bg2"], np.float32)).reshape(1, 1)

    adjT = [np.asarray(inputs["adj1"]).T, np.asarray(inputs["adj2"]).T]
    in_maps = []
    for c in range(cfg.NC):
        m = dict(shared)
        sl = slice(EL * c, EL * (c + 1))
        m["xTl"] = np.ascontiguousarray(shared["xT"][:, sl])
        for a in range(2):
            mt = adjT[a][:, sl].astype(f16)           # [E, EL]
            m[f"mask{a}"] = np.ascontiguousarray(mt.reshape(NFC, 128, EL))
        in_maps.append(m)
    return in_maps


_CACHE = {}


def _get_nc(cfg, reps=1):
    key = (cfg.E, cfg.NC, cfg.CS, reps)
    if key not in _CACHE:
        _CACHE[key] = build_nc(cfg, reps)
    return _CACHE[key]


def kernel(**inputs):
    cfg = Cfg()
    nc = _get_nc(cfg)
    in_maps = host_prep(cfg, inputs)
    res = run_bass_kernel_spmd(nc, in_maps, core_ids=list(range(cfg.NC)))
    logits = np.concatenate(
        [np.ascontiguousarray(res.results[c]["logitsT"].T) for c in range(cfg.NC)],
        axis=0)
    alpha_mean = np.float32(
        sum(float(res.results[c]["alpha_part"][0, 0]) for c in range(cfg.NC))
        / cfg.E)
    return logits.astype(np.float32), alpha_mean


# revision 41
# speedup vs baseline: 15.5329x; 2.2584x over previous
"""Trainium2 distributed kernel for nn_AdaptiveHopModel (gnn_message_passing).

Model: two masked edge-attention blocks over E=4096 edges (D=256, H=4), an
adaptive per-edge hop gate alpha = sigmoid(MLP(x)), blended output fed
through a small classifier; returns (logits [E,16], alpha.mean()).

Sharding: query-edge rows (E) split across 8 NeuronCores (512 rows/core),
weights replicated, K/V computed for all E on every core (no collectives —
measured AllGather cost in this environment dwarfs the replicated compute).
Activations are kept in a transposed [feature, edge] layout so every x@W
matmul maps directly onto the TensorEngine.  The masked softmax streams:
PE scoresT -> ScalarE exp (PSUM->SBUF, fp16) -> DVE mask-multiply ->
PE attn@V with a ones-column in V producing row-sums for free;
normalization happens on the small [65, e] output.

Masks are pre-transposed/tiled on the host as fp16 {0,1}; weights arrive
packed into three wide tensors (3 DMAs); alpha.mean() is assembled on the
host from per-core partials; logits come back transposed [16, 512] per
core and are transposed on the host.
"""

import sys

sys.path.insert(0, "/opt/trn_rl_repo")

import numpy as np

import concourse.bass as bass
import concourse.bacc as bacc
import concourse.tile as tile
import concourse.mybir as mybir
from concourse.bass_utils import run_bass_kernel_spmd

F16 = mybir.dt.float16
F32 = mybir.dt.float32
AF = mybir.ActivationFunctionType
MULT = mybir.AluOpType.mult
ADD = mybir.AluOpType.add

SQRT_2_OVER_PI = 0.7978845608028654
GELU_C = 0.044715


class Cfg:
    def __init__(self, E=4096, n_cores=8, CS=2):
        self.E = E                      # total edges
        self.D = 256                    # edge feature dim
        self.H = 4                      # heads
        self.DH = 64                    # head dim
        self.NC = n_cores
        self.EL = E // n_cores          # local query rows per core
        self.NFC = E // 128             # f-chunks of 128 key rows
        self.CS = CS                    # f-chunks per softmax stripe
        self.NCLS = 16
        self.DG = 128                   # gate hidden dim
        self.n_stripes = (self.NFC + CS - 1) // CS


def _emit_core_program(nc, tc, ctx, io, cfg, rep, dbg=False):
    """Emit the whole per-core program (one repetition)."""
    E, EL, NFC, CS = cfg.E, cfg.EL, cfg.NFC, cfg.CS
    R = f"r{rep}"

    konst = ctx.enter_context(tc.tile_pool(name=f"konst{R}", bufs=1))
    ps512 = ctx.enter_context(tc.tile_pool(name=f"ps512{R}", bufs=2, space="PSUM"))
    tmp = ctx.enter_context(tc.tile_pool(name=f"tmp{R}", bufs=2))
    small = ctx.enter_context(tc.tile_pool(name=f"small{R}", bufs=1))
    apool = ctx.enter_context(tc.tile_pool(name=f"attn{R}", bufs=1))
    scp = ctx.enter_context(tc.tile_pool(name=f"scp{R}", bufs=2, space="PSUM"))
    stream = ctx.enter_context(tc.tile_pool(name=f"stream{R}", bufs=3))

    def ktile(nm, shape, dtype):
        return konst.tile(shape, dtype, name=nm, tag=nm)

    def load_rows(nm, src, nrow=128, nsplit=1, eng=None):
        eng = eng or nc.sync
        n = io[src].shape[0] // nrow
        ts = []
        for d in range(n):
            t = ktile(f"{nm}{d}", [nrow, io[src].shape[1]], io[src].dtype)
            if nsplit == 1:
                eng.dma_start(t[:], io[src][nrow * d:nrow * (d + 1), :])
            else:
                step = io[src].shape[1] // nsplit
                for i in range(nsplit):
                    eng.dma_start(
                        t[:, step * i:step * (i + 1)],
                        io[src][nrow * d:nrow * (d + 1), step * i:step * (i + 1)])
            ts.append(t)
        return ts

    # ---------------- constant loads (packed) ----------------
    xTl = load_rows("xTl", "xTl")
    xpool_cm = tc.tile_pool(name=f"xpool{R}", bufs=1)
    xpool = xpool_cm.__enter__()
    xT = []
    for d in range(2):
        t = xpool.tile([128, E], F16, name=f"xT{d}", tag=f"xT{d}")
        for i in range(4):
            nc.sync.dma_start(t[:, (E // 4) * i:(E // 4) * (i + 1)],
                              io["xT"][128 * d:128 * (d + 1),
                                       (E // 4) * i:(E // 4) * (i + 1)])
        xT.append(t)

    wpack = ktile("wpack", [128, io["wpack"].shape[1]], F16)
    nc.sync.dma_start(wpack[:], io["wpack"][:])
    bpack = ktile("bpack", [128, io["bpack"].shape[1]], F32)
    nc.sync.dma_start(bpack[:], io["bpack"][:])
    rpack = ktile("rpack", [1, io["rpack"].shape[1]], F16)
    nc.sync.dma_start(rpack[:], io["rpack"][:])
    bg2h = ktile("bg2h", [1, 1], F32)
    nc.sync.dma_start(bg2h[:], io["bg2h"][:])

    col = 0

    def wslice(width):
        nonlocal col
        s = wpack[:, col:col + width]
        col += width
        return s

    wqkv, wv_aug, wo = [], [], []
    for a in range(2):
        wqkv.append([wslice(768) for _ in range(2)])
        wv_aug.append([wslice(260) for _ in range(2)])
        wo.append([wslice(256) for _ in range(2)])
    wg1 = [wslice(cfg.DG) for _ in range(2)]
    wc1 = [wslice(256) for _ in range(2)]
    wc2 = [wslice(cfg.NCLS) for _ in range(2)]
    wg2h = wslice(1)

    bcol = 0

    def bslice(n=1):
        nonlocal bcol
        s = bpack[:, bcol:bcol + n]
        bcol += n
        return s

    bq, bk = [], []
    for a in range(2):
        bq.append([bslice() for _ in range(2)])
        bk.append([bslice() for _ in range(2)])
    bg1 = bslice()
    bc1 = [bslice() for _ in range(2)]
    bc2 = bslice()[0:cfg.NCLS, :]

    rcol = 0

    def rslice(n):
        nonlocal rcol
        s = rpack[:, rcol:rcol + n]
        rcol += n
        return s

    bv_aug, bo = [], []
    for a in range(2):
        bv_aug.append(rslice(260))
        bo.append(rslice(256))

    ones_row = ktile("ones_row", [1, 128], F16)
    nc.vector.memset(ones_row[:], 1.0)

    def pstile():
        return ps512.tile([128, 512], F32, name="ps", tag="ps")

    def gelu2(z_ps, bias_ap, out_f16, P, N):
        """out = z*(1+tanh(c*(z+GELU_C*z^3))) = 2*gelu(z), z = z_ps + bias."""
        z = tmp.tile([128, 512], F32, name="gelu_z", tag="gelu_z")
        nc.vector.tensor_scalar(z[0:P, 0:N], z_ps, bias_ap, None, op0=ADD)
        u = tmp.tile([128, 512], F32, name="gelu_u", tag="gelu_t")
        nc.vector.tensor_tensor(u[0:P, 0:N], z[0:P, 0:N], z[0:P, 0:N], op=MULT)
        w = tmp.tile([128, 512], F32, name="gelu_w", tag="gelu_t")
        nc.vector.tensor_scalar(w[0:P, 0:N], u[0:P, 0:N], GELU_C, 1.0, op0=MULT, op1=ADD)
        t2 = tmp.tile([128, 512], F32, name="gelu_t2", tag="gelu_t")
        nc.vector.tensor_tensor(t2[0:P, 0:N], z[0:P, 0:N], w[0:P, 0:N], op=MULT)
        th = tmp.tile([128, 512], F32, name="gelu_th", tag="gelu_t")
        nc.scalar.activation(th[0:P, 0:N], t2[0:P, 0:N], AF.Tanh, scale=SQRT_2_OVER_PI)
        nc.vector.scalar_tensor_tensor(out_f16, th[0:P, 0:N], 1.0, z[0:P, 0:N],
                                       op0=ADD, op1=MULT)

    def stile(nm, shape, dtype):
        return small.tile(shape, dtype, name=nm, tag=nm)

    def atile(nm, shape, dtype):
        return apool.tile(shape, dtype, name=nm, tag=nm)

    # ---------------- q/k/v for both attentions ----------------
    qTs, kTs, vss = [], [], []
    for a in range(2):
        qT = [atile(f"qT{m}_{a}", [128, EL], F16) for m in range(2)]
        for m in range(2):
            ps = pstile()
            psv = ps[:, 0:EL]
            nc.tensor.matmul(psv, wqkv[a][0][:, 128 * m:128 * (m + 1)],
                             xTl[0][:], start=True, stop=False)
            nc.tensor.matmul(psv, wqkv[a][1][:, 128 * m:128 * (m + 1)],
                             xTl[1][:], start=False, stop=True)
            nc.vector.tensor_scalar(qT[m][:], psv, bq[a][m], None, op0=ADD)
        kT = [atile(f"kT{m}_{a}", [128, E], F16) for m in range(2)]
        for m in range(2):
            for nn in range(E // 512):
                ps = pstile()
                sl = slice(512 * nn, 512 * (nn + 1))
                nc.tensor.matmul(ps[:], wqkv[a][0][:, 256 + 128 * m:256 + 128 * (m + 1)],
                                 xT[0][:, sl], start=True, stop=False)
                nc.tensor.matmul(ps[:], wqkv[a][1][:, 256 + 128 * m:256 + 128 * (m + 1)],
                                 xT[1][:, sl], start=False, stop=True)
                nc.vector.tensor_scalar(kT[m][:, sl], ps[:], bk[a][m], None, op0=ADD)
        v_store = atile(f"v_store_{a}", [128, NFC * 260], F16)
        for t in range(NFC):
            ps = pstile()
            psv = ps[:, 0:260]
            fsl = slice(128 * t, 128 * (t + 1))
            nc.tensor.matmul(psv, xT[0][:, fsl], wv_aug[a][0], start=True, stop=False)
            nc.tensor.matmul(psv, xT[1][:, fsl], wv_aug[a][1], start=False, stop=False)
            nc.tensor.matmul(psv, ones_row[:], bv_aug[a], start=False, stop=True)
            nc.vector.tensor_copy(v_store[:, 260 * t:260 * (t + 1)], psv)
        qTs.append(qT)
        kTs.append(kT)
        vss.append(v_store)

    xpool_cm.__exit__(None, None, None)

    # ---------------- gate path ----------------
    hg_ps = pstile()
    hg_psv = hg_ps[0:cfg.DG, 0:EL]
    nc.tensor.matmul(hg_psv, wg1[0], xTl[0][:], start=True, stop=False)
    nc.tensor.matmul(hg_psv, wg1[1], xTl[1][:], start=False, stop=True)
    hg = stile("hg", [cfg.DG, EL], F16)  # = 2*gelu(gate pre-act)
    gelu2(hg_psv, bg1, hg[:], cfg.DG, EL)
    al_ps = pstile()
    nc.tensor.matmul(al_ps[0:1, 0:EL], wg2h, hg[:], start=True, stop=True)
    tha = stile("tha", [1, EL], F32)
    # sigmoid(l) = 0.5 + 0.5*tanh(0.5*l); wg2h has 0.25*Wg2 folded in
    nc.scalar.activation(tha[:], al_ps[0:1, 0:EL], AF.Tanh, scale=1.0, bias=bg2h[:])
    alpha = stile("alpha", [1, EL], F32)
    nc.vector.tensor_scalar(alpha[:], tha[:], 0.5, 0.5, op0=MULT, op1=ADD)
    ac = stile("ac", [1, EL], F32)
    nc.vector.tensor_scalar(ac[:], alpha[:], -1.0, 1.0, op0=MULT, op1=ADD)
    alpha16 = stile("alpha16", [1, EL], F16)
    nc.vector.tensor_copy(alpha16[:], alpha[:])
    ac16 = stile("ac16", [1, EL], F16)
    nc.vector.tensor_copy(ac16[:], ac[:])
    apart = stile("apart", [1, 1], F32)
    nc.vector.reduce_sum(apart[:], alpha[:], axis=mybir.AxisListType.X)
    nc.gpsimd.dma_start(io["alpha_part"][:], apart[:])

    ab = stile("ab", [128, EL], F32)
    ab_ps = pstile()
    nc.tensor.matmul(ab_ps[:, 0:EL], ones_row[:], alpha16[:], start=True, stop=True)
    nc.vector.tensor_copy(ab[:], ab_ps[:, 0:EL])
    acb = stile("acb", [128, EL], F32)
    acb_ps = pstile()
    nc.tensor.matmul(acb_ps[:, 0:EL], ones_row[:], ac16[:], start=True, stop=True)
    nc.vector.tensor_copy(acb[:], acb_ps[:, 0:EL])

    # ---------------- attention head loops ----------------
    stream = ctx.enter_context(tc.tile_pool(name=f"stream{R}", bufs=3))
    P_sb = []
    for a in range(2):
        qT, kT, v_store = qTs[a], kTs[a], vss[a]
        outT = [atile(f"outT{m}_{a}", [128, EL], F16) for m in range(2)]
        rb = [atile(f"rb{m}_{a}", [128, EL], F32) for m in range(2)]
        for pair in range(2):
            avacc = [ps512.tile([128, 512], F32, name=f"avacc{hh}", tag="av")
                     for hh in range(2)]
            for s in range(cfg.n_stripes):
                w_s = min(CS, NFC - CS * s)
                Ns = EL * w_s
                mt = stream.tile([128, EL * CS], F16, name="mt", tag="mask")
                for j in range(w_s):
                    nc.sync.dma_start(mt[:, EL * j:EL * (j + 1)],
                                      io[f"mask{a}"][CS * s + j])
                for hh in range(2):
                    h = 2 * pair + hh
                    mi, po = h // 2, 64 * (h % 2)
                    sc = scp.tile([128, EL * CS], F32, name="sc", tag="sc")
                    for j in range(w_s):
                        t = CS * s + j
                        nc.tensor.matmul(
                            sc[0:128, EL * j:EL * j + EL],
                            kT[mi][po:po + 64, 128 * t:128 * (t + 1)],
                            qT[mi][po:po + 64, :],
                            start=True, stop=True)
                    pm = stream.tile([128, EL * CS], F16, name="pm", tag="pm", bufs=4)
                    nc.scalar.activation(pm[:, 0:Ns], sc[:, 0:Ns], AF.Exp, scale=0.125)
                    pmm = stream.tile([128, EL * CS], F16, name="pmm", tag="pmm", bufs=4)
                    nc.vector.tensor_tensor(pmm[:, 0:Ns], pm[:, 0:Ns], mt[:, 0:Ns], op=MULT)
                    for j in range(w_s):
                        t = CS * s + j
                        nc.tensor.matmul(
                            avacc[hh][0:65, 0:EL],
                            v_store[:, 260 * t + 65 * h:260 * t + 65 * h + 65],
                            pmm[:, EL * j:EL * j + EL],
                            start=(t == 0), stop=(t == NFC - 1))
            # normalize: out[d, e] = avacc[d, e] * (1 / avacc[64, e])
            mi = pair
            rb_ps = scp.tile([128, EL * CS], F32, name="rb_ps", tag="sc")
            for hh in range(2):
                rcp = stile(f"rcp{a}_{pair}_{hh}", [1, EL], F32)
                nc.vector.reciprocal(rcp[:], avacc[hh][64:65, 0:EL])
                rcph = stile(f"rcph{a}_{pair}_{hh}", [1, EL], F16)
                nc.vector.tensor_copy(rcph[:], rcp[:])
                nc.tensor.matmul(rb_ps[64 * hh:64 * hh + 64, 0:EL],
                                 ones_row[0:1, 0:64], rcph[:],
                                 start=True, stop=True)
            nc.vector.tensor_copy(rb[mi][:], rb_ps[:, 0:EL])
            for hh in range(2):
                po = 64 * hh
                nc.vector.tensor_tensor(outT[mi][po:po + 64, :],
                                        avacc[hh][0:64, 0:EL],
                                        rb[mi][po:po + 64, :], op=MULT)

        # ---- P = Wo^T @ outT, premultiplied by blend coefficient ----
        Pm = []
        cb = acb if a == 0 else ab
        for m in range(2):
            ps = pstile()
            psv = ps[:, 0:EL]
            nc.tensor.matmul(psv, wo[a][0][:, 128 * m:128 * (m + 1)], outT[0][:],
                             start=True, stop=False)
            nc.tensor.matmul(psv, wo[a][1][:, 128 * m:128 * (m + 1)], outT[1][:],
                             start=False, stop=True)
            p_sb = tmp.tile([128, 512], F32, name=f"P{a}_{m}", tag=f"P{a}_{m}")
            nc.vector.tensor_tensor(p_sb[:, 0:EL], psv, cb[:], op=MULT)
            Pm.append(p_sb)
        P_sb.append(Pm)

    # ---------------- blend + classifier ----------------
    blended = [stile(f"blend{m}", [128, EL], F16) for m in range(2)]
    for m in range(2):
        bb = pstile()
        bbv = bb[:, 0:EL]
        nc.tensor.matmul(bbv, bo[0][0:1, 128 * m:128 * (m + 1)], ac16[:], start=True, stop=False)
        nc.tensor.matmul(bbv, bo[1][0:1, 128 * m:128 * (m + 1)], alpha16[:], start=False, stop=True)
        t3 = tmp.tile([128, 512], F32, name="bl_t3", tag="bl_t3")
        nc.vector.tensor_tensor(t3[:, 0:EL], P_sb[0][m][:, 0:EL], P_sb[1][m][:, 0:EL], op=ADD)
        nc.vector.tensor_tensor(blended[m][:], t3[:, 0:EL], bbv, op=ADD)

    chT = [stile(f"ch{m}", [128, EL], F16) for m in range(2)]
    for m in range(2):
        ps = pstile()
        psv = ps[:, 0:EL]
        nc.tensor.matmul(psv, wc1[0][:, 128 * m:128 * (m + 1)], blended[0][:], start=True, stop=False)
        nc.tensor.matmul(psv, wc1[1][:, 128 * m:128 * (m + 1)], blended[1][:], start=False, stop=True)
        gelu2(psv, bc1[m], chT[m][:], 128, EL)
    # logitsT = Wc2'^T @ chT + bc2, with the 0.5 of gelu2 folded into Wc2'
    lp = pstile()
    lpv = lp[0:cfg.NCLS, 0:EL]
    nc.tensor.matmul(lpv, wc2[0], chT[0][:], start=True, stop=False)
    nc.tensor.matmul(lpv, wc2[1], chT[1][:], start=False, stop=True)
    logT = stile("logT", [cfg.NCLS, EL], F32)
    nc.vector.tensor_scalar(logT[:], lpv, bc2, None, op0=ADD)
    nc.sync.dma_start(io["logitsT"][:], logT[:])


def build_nc(cfg, reps=1, dbg=False):
    nc = bacc.Bacc("TRN2", target_bir_lowering=False, debug=False,
                   num_devices=cfg.NC)
    io = {}

    def din(name, shape, dtype):
        io[name] = nc.dram_tensor(name, shape, dtype, kind="ExternalInput").ap()

    E = cfg.E
    din("xT", [256, E], F16)
    din("xTl", [256, cfg.EL], F16)
    for a in range(2):
        din(f"mask{a}", [cfg.NFC, 128, cfg.EL], F16)
    WCOLS = 2 * (2 * 768 + 2 * 260 + 2 * 256) + 2 * cfg.DG + 2 * 256 + 2 * cfg.NCLS + 1
    din("wpack", [128, WCOLS], F16)
    din("bpack", [128, 12], F32)
    din("rpack", [1, 2 * (260 + 256)], F16)
    din("bg2h", [1, 1], F32)
    io["logitsT"] = nc.dram_tensor("logitsT", [cfg.NCLS, cfg.EL], F32,
                                   kind="ExternalOutput").ap()
    io["alpha_part"] = nc.dram_tensor("alpha_part", [1, 1], F32,
                                      kind="ExternalOutput").ap()
    if dbg:
        for nm, shape, dt in [
            ("d_ab", [128, cfg.EL], F32), ("d_alpha", [1, cfg.EL], F32),
            ("d_pmm", [128, cfg.EL], F16), ("d_pm", [128, cfg.EL], F16),
            ("d_mt", [128, cfg.EL], F16), ("d_avacc", [65, cfg.EL], F32),
            ("d_rb", [128, cfg.EL], F32), ("d_outT", [128, cfg.EL], F16),
            ("d_P", [128, cfg.EL], F32), ("d_qT", [128, cfg.EL], F16),
            ("d_kT", [128, cfg.EL], F16), ("d_v", [128, 260], F16),
            ("d_bb", [128, cfg.EL], F32), ("d_t3", [128, cfg.EL], F32),
            ("d_blended", [128, cfg.EL], F16), ("d_zc", [128, cfg.EL], F32),
            ("d_ch", [128, cfg.EL], F16),
            ("d_t1", [128, cfg.EL], F32), ("d_t2", [128, cfg.EL], F32),
            ("d_ab2", [128, cfg.EL], F32), ("d_acb2", [128, cfg.EL], F32),
            ("d_P00", [128, cfg.EL], F32), ("d_P10", [128, cfg.EL], F32),
            ("d_qps", [128, cfg.EL], F32),
            ("d_hg", [128, cfg.EL], F16), ("d_tha", [1, cfg.EL], F32),
            ("d_hgps", [128, cfg.EL], F32),
        ]:
            io[nm] = nc.dram_tensor(nm, shape, dt, kind="ExternalOutput").ap()

    from contextlib import ExitStack
    with tile.TileContext(nc) as tc:
        for rep in range(reps):
            with ExitStack() as ctx:
                _emit_core_program(nc, tc, ctx, io, cfg, rep, dbg=dbg)
    nc.compile()
    return nc


def host_prep(cfg, inputs):
    """Build the per-core input maps from the full problem inputs."""
    E, EL, NFC = cfg.E, cfg.EL, cfg.NFC
    f16 = np.float16
    x = np.asarray(inputs["edge_features"], np.float32)
    xT = np.ascontiguousarray(x.T).astype(f16)
    shared = {"xT": xT}
    wcols, bcols, rcols = [], [], []
    for a, (wq, bqv, wo_, bo_) in enumerate([
        (inputs["Wqkv1"], inputs["bqkv1"], inputs["Wo1"], inputs["bo1"]),
        (inputs["Wqkv2"], inputs["bqkv2"], inputs["Wo2"], inputs["bo2"]),
    ]):
        wq = np.asarray(wq, np.float32)
        bqv = np.asarray(bqv, np.float32)
        wo_ = np.asarray(wo_, np.float32)
        bo_ = np.asarray(bo_, np.float32)
        wv_aug = np.zeros((256, 260), np.float32)
        bv_aug = np.zeros((1, 260), np.float32)
        for h in range(4):
            wv_aug[:, 65 * h:65 * h + 64] = wq[:, 512 + 64 * h:512 + 64 * (h + 1)]
            bv_aug[0, 65 * h:65 * h + 64] = bqv[512 + 64 * h:512 + 64 * (h + 1)]
            bv_aug[0, 65 * h + 64] = 1.0
        wcols += [wq[0:128], wq[128:256], wv_aug[0:128], wv_aug[128:256],
                  wo_[0:128], wo_[128:256]]
        bcols += [bqv[0:128, None], bqv[128:256, None],
                  bqv[256:384, None], bqv[384:512, None]]
        rcols += [bv_aug, bo_.reshape(1, 256)]
    wg1 = np.asarray(inputs["Wg1"], np.float32)
    wc1 = np.asarray(inputs["Wc1"], np.float32)
    wc2h = 0.5 * np.asarray(inputs["Wc2"], np.float32)
    wg2h = 0.25 * np.asarray(inputs["Wg2"], np.float32)
    wcols += [wg1[0:128], wg1[128:256], wc1[0:128], wc1[128:256],
              wc2h[0:128], wc2h[128:256], wg2h[0:128]]
    bc2c = np.zeros((128, 1), np.float32)
    bc2c[0:16, 0] = np.asarray(inputs["bc2"], np.float32)
    bcols += [np.asarray(inputs["bg1"], np.float32).reshape(128, 1),
              np.asarray(inputs["bc1"], np.float32)[0:128, None],
              np.asarray(inputs["bc1"], np.float32)[128:256, None],
              bc2c]
    shared["wpack"] = np.ascontiguousarray(np.concatenate(wcols, axis=1)).astype(f16)
    shared["bpack"] = np.ascontiguousarray(np.concatenate(bcols, axis=1)).astype(np.float32)
    shared["rpack"] = np.ascontiguousarray(np.concatenate(rcols, axis=1)).astype(f16)
    shared["bg2h"] = (0.5 * np.asarray(inputs["bg2"], np.float32)).reshape(1, 1)

    adjT = [np.asarray(inputs["adj1"]).T, np.asarray(inputs["adj2"]).T]
    in_maps = []
    for c in range(cfg.NC):
        m = dict(shared)
        sl = slice(EL * c, EL * (c + 1))
        m["xTl"] = np.ascontiguousarray(shared["xT"][:, sl])
        for a in range(2):
            mt = adjT[a][:, sl].astype(f16)           # [E, EL]
            m[f"mask{a}"] = np.ascontiguousarray(mt.reshape(NFC, 128, EL))
        in_maps.append(m)
    return in_maps


_CACHE = {}


def _get_nc(cfg, reps=1):
    key = (cfg.E, cfg.NC, cfg.CS, reps)
    if key not in _CACHE:
        _CACHE[key] = build_nc(cfg, reps)
    return _CACHE[key]


def kernel(**inputs):
    cfg = Cfg()
    nc = _get_nc(cfg)
    in_maps = host_prep(cfg, inputs)
    res = run_bass_kernel_spmd(nc, in_maps, core_ids=list(range(cfg.NC)))
    logits = np.concatenate(
        [np.ascontiguousarray(res.results[c]["logitsT"].T) for c in range(cfg.NC)],
        axis=0)
    alpha_mean = np.float32(
        sum(float(res.results[c]["alpha_part"][0, 0]) for c in range(cfg.NC))
        / cfg.E)
    return logits.astype(np.float32), alpha_mean
